# revision 27
# baseline (speedup 1.0000x reference)
"""BertSelfAttention (B=4, S=4096, D=512) on 8 TRN2 NeuronCores.

Sharding: core c handles batch b = c//2 and query-row half h = c%2
(2048 q rows). K/V are computed on-core for the full 4096 keys of that
batch (halves exchanged within each core pair), avoiding big collectives.

Layout trick: everything is computed transposed so no on-device
transposes are needed:
  QT[e, q] = Wq @ x.T          (lhsT = WqT chunks, rhs = xT chunks)
  KT[e, k] = Wk @ x.T
  V [k, e] = x @ Wv.T          (lhsT = xT chunks,  rhs = WvT)
  ST[k, q] = K Q.T             (lhsT = KT chunks,  rhs = QT)   -> exp -> PT
  OT[e, q] = V.T P.T           (lhsT = V chunks,   rhs = PT)
Softmax runs without max-subtraction (scores are ~N(0, 0.3^2), so exp
cannot overflow and the result is mathematically identical).

Precision: projections run in bf16 (fp32 PSUM); Q/K/V/P are quantized
to fp8e4 (e4m3) and the two big matmuls run as fp8 DoubleRow matmuls
(256-deep contraction per pass = 2x bf16 MAC rate on HW). The rank-1
component of the V-quantization error (softmax rows sum to one) is
corrected by adding (colsum(V_f32) - colsum(V_fp8))/S to the output
bias; colsum(V_f32) accumulates on the DVE during the V projection and
colsum(V_fp8) is an all-ones fp8 matmul, so the correction costs ~2us.

Schedule (trace-driven rewrite of the 209.6us baseline; ~205.7us):
- v1 put the softmax row-sums on the DVE (2 adds/pair): the DVE fell
  ~9us behind the exp stream, and its rowsum -> cast -> ones-matmul ->
  3.4us-reciprocal chain stalled the PE twice (8.9us + 4.3us) with
  clock-droop penalties on each restart. Here the row-sums are 16
  all-ones fp8 DoubleRow matmuls per chunk (+216ns/pair on the PE,
  partition-replicated result in one PSUM bank), emitted right after
  the chunk's partner score block; the reciprocal follows immediately
  and the DVE stays ~60% idle. (GpSimd tensor ops measured 3-15x
  slower than the cost model — eff ~0.05 for tensor_scalar — so
  offloading element work to Pool is a dead end on this HW.)
- AV matmuls for q-chunk qc are interleaved per-pair into the NEXT
  chunk's partner-half score block (8 av matmuls per score slot,
  e-major), so the PE runs back-to-back instead of alternating
  exp-paced score stretches with pure-AV stretches. Evacuation:
  DVE normalize-multiply, ACT bias-add (emitted after the slot's exp
  so the ACT queue stays exps-in-order), DMA out on alternating
  queues; the final chunk evacuates in 256-col halves to shorten the
  drain tail.
- K and V projections are fused per x column-chunk (4 KT e-tiles + 4 V
  k-tiles per chunk, ~7us of PE work per 512KB of arriving x), so the
  PE rides just behind the input DMA stream instead of stalling on it.
  Q chunks 1-3 ride inside the local score blocks (2 matmuls/slot,
  DVE evacuation); only K, V and Q-chunk-0 gate attention start.
- Inputs load via the 3 DMA-capable queues (SP/GpSimd/ACT): wk+wv
  first, then x column-chunks striped in consumption order; ~96
  throwaway matmuls warm the PE clock gate during the DMA wait.
- Collective order matters: KT, then V, then the tiny s exchange —
  inserting s before V costs a full collective-startup latency on the
  1MB V gather and starves the AV phase (measured +37us).
- PSUM: 2x2-bank score tiles, a 3-buffer [128,512] pool shared by AV
  accumulators and Q-projection tiles, and 1 rowsum bank (8/8 banks).
"""

import sys

for _p in ("/opt/trn_rl_repo", "/root/.axon_site/_ro/trn_rl_repo"):
    if _p not in sys.path:
        sys.path.append(_p)

import numpy as np
import ml_dtypes

B, S, D = 4, 4096, 512
NCORES = 8
SQ = S // 2  # query rows per core
P = 128
NQ = 512  # q-chunk width (moving free dim)
DT = D // P  # 4 contraction chunks for d
ET = D // P  # 4 e tiles
KTI = S // P  # 32 k tiles
QC = SQ // NQ  # 4 q chunks per core
HKT = KTI // 2  # 16 local k-tiles per core
HS = S // 2  # 2048 local keys per core
NPAIR = HKT // 2  # 8 k-tile pairs per half-block
SCALE = 1.0 / float(np.sqrt(np.float32(D)))
NWARM = 96

_CACHE = {}


def _split_excess_waits(nc, mybir, max_waits=1):
    """This walrus build rejects instructions carrying more than a couple of
    sync waits. Cap every instruction at `max_waits`, spilling the rest onto
    same-engine InstNoOps inserted immediately before it (equivalent
    semantics: the engine's stream stalls at the nop instead)."""
    for f in nc.m.functions:
        for bb in f.blocks:
            old = list(bb.instructions)
            new = []
            for inst in old:
                si = inst.sync_info
                waits = list(si.on_wait) if si is not None and si.on_wait else []
                if len(waits) > max_waits:
                    keep = waits[-max_waits:]
                    excess = waits[:-max_waits]
                    for i in range(0, len(excess), max_waits):
                        nop = mybir.InstNoOp(
                            name=f"waitnop-{nc.next_id()}", ins=[], outs=[]
                        )
                        nop.engine = inst.engine
                        nop.sync_info = mybir.SyncInfo(
                            on_wait=excess[i : i + max_waits], on_update=[]
                        )
                        new.append(nop)
                    inst.sync_info = mybir.SyncInfo(
                        on_wait=keep,
                        on_update=list(si.on_update) if si.on_update else [],
                    )
                new.append(inst)
            if len(new) != len(old):
                bb.instructions[:] = new


def _build_nc():
    import concourse.bass as bass
    import concourse.mybir as mybir
    import concourse.tile as tile
    from contextlib import ExitStack

    bf = mybir.dt.bfloat16
    f32 = mybir.dt.float32
    f8 = mybir.dt.float8e4
    AF = mybir.ActivationFunctionType
    DR = mybir.MatmulPerfMode.DoubleRow
    ALU = mybir.AluOpType

    u32 = mybir.dt.uint32
    nc = bass.Bass()
    xT = nc.declare_dram_parameter("xT", [D, SQ], bf, isOutput=False)
    # Weights host-retiled to [P, DT*D]: w_re[p, d*D+j] = wT[d*P+p, j], so
    # each partition's data is one contiguous 4KB run and the whole matrix
    # loads as a single high-throughput DMA.
    wqT = nc.declare_dram_parameter("wqT", [P, DT * D], bf, isOutput=False)
    wkT = nc.declare_dram_parameter("wkT", [P, DT * D], bf, isOutput=False)
    wvT = nc.declare_dram_parameter("wvT", [P, DT * D], bf, isOutput=False)
    bqp = nc.declare_dram_parameter("bq", [P, ET], f32, isOutput=False)
    bkp = nc.declare_dram_parameter("bk", [P, ET], f32, isOutput=False)
    bvp = nc.declare_dram_parameter("bv", [P, ET], f32, isOutput=False)
    # Host-computed row bases into the AllGather outputs for the PARTNER
    # half (rank-dependent: (1-h)*512 + e*128 for KT, (1-h)*2048 + j*128
    # for V). Drives dynamic (register-offset) DMAs.
    poffp = nc.declare_dram_parameter("poff", [1, 2], u32, isOutput=False)
    ot = nc.declare_dram_parameter("ot", [D, SQ], f32, isOutput=True)

    with tile.TileContext(nc) as tc, ExitStack() as ctx:
        const_pool = ctx.enter_context(tc.tile_pool(name="const", bufs=1))
        persist = ctx.enter_context(tc.tile_pool(name="persist", bufs=1))
        outp = ctx.enter_context(tc.tile_pool(name="outp", bufs=2))
        xin_pool = ctx.enter_context(tc.tile_pool(name="xin", bufs=1))

        ones = const_pool.tile([P, P], bf, tag="ones")
        nc.vector.memset(ones, 1.0)
        ones_f8 = const_pool.tile([P, 1], f8, tag="ones8")
        nc.gpsimd.memset(ones_f8, 1.0)
        # Wide all-ones fp8 stationary for the DoubleRow rowsum matmuls
        # (partition-reduces a PT pair and replicates across partitions).
        ones_f8w = const_pool.tile([P, 2, P], f8, tag="ones8w")
        nc.gpsimd.memset(ones_f8w, 1.0)
        bq_sb = const_pool.tile([P, ET], f32, tag="bq")
        bk_sb = const_pool.tile([P, ET], f32, tag="bk")
        bv_sb = const_pool.tile([P, ET], f32, tag="bv")
        wq_sb = const_pool.tile([P, DT, D], bf, tag="wq", name="wq")
        wk_sb = const_pool.tile([P, DT, D], bf, tag="wk", name="wk")
        wv_sb = const_pool.tile([P, DT, D], bf, tag="wv", name="wv")
        # fp8 operand tiles for the DoubleRow matmuls; contraction-paired
        # chunks live in dim 1 so [:, e:e+2, cols] is a valid 3D AP.
        qt_sb = persist.tile([P, ET, SQ], f8, tag="qt", name="qt")
        # K/V k-order per core: [my half, partner half]. Separate tiles per
        # half so partner DMA-writes create no false deps on local reads.
        kt_loc = persist.tile([P, ET, HS], f8, tag="ktl", name="ktl")
        kt_rem = persist.tile([P, ET, HS], f8, tag="ktr", name="ktr")
        v_loc = persist.tile([P, HKT, D], f8, tag="vl", name="vl")
        v_rem = persist.tile([P, HKT, D], f8, tag="vr", name="vr")
        poff_sb = const_pool.tile([1, 2], u32, tag="poff")
        bias2 = const_pool.tile([P, ET], f32, tag="bias2")

        # ---- Phase 1+2: load inputs on 4 DMA queues, project K + local V,
        # AllGather the K/V halves within each core pair, project Q chunk 0.
        with (
            tc.tile_pool(name="psA", bufs=4, space="PSUM") as psA,
            tc.tile_pool(name="dram", bufs=1, space="DRAM") as dram,
        ):
            ktl_d = dram.tile([ET * P, HS], f8, tag="ktl_d")
            ktg_d = dram.tile([2 * ET * P, HS], f8, tag="ktg_d")
            vl_d = dram.tile([HKT * P, D], f8, tag="vl_d")
            vg_d = dram.tile([2 * HKT * P, D], f8, tag="vg_d")
            sl_d = dram.tile([1, D], f32, tag="sl_d")
            sg_d = dram.tile([2, D], f32, tag="sg_d")

            x_sb = [xin_pool.tile([P, HS], bf, tag=f"x{d}", name=f"x{d}") for d in range(DT)]
            # 3 parallel hardware DMA queues (SP, GpSimd, ACT — the only
            # DMA-capable engines); per queue: wk chunks first (the first
            # projection needs them), then x column-chunks in first-consumer
            # order, then the later-needed weights/biases.
            qeng = [nc.sync, nc.gpsimd, nc.scalar]
            # Whole-tensor transfers with 4KB-contiguous runs per partition:
            # wk + wv upfront (the fused K/V projection needs both), the
            # four x d-chunks (full 2048-col rows), then the late-needed wq.
            qeng[0].dma_start(
                out=wk_sb, in_=wkT[:, :].rearrange("p (d c) -> p d c", d=DT)
            )
            qeng[1].dma_start(
                out=wv_sb, in_=wvT[:, :].rearrange("p (d c) -> p d c", d=DT)
            )
            for d in range(DT):
                qeng[(2 + d) % 3].dma_start(
                    out=x_sb[d], in_=xT[d * P : (d + 1) * P, :]
                )
            qeng[1].dma_start(
                out=wq_sb, in_=wqT[:, :].rearrange("p (d c) -> p d c", d=DT)
            )
            qeng[1].dma_start(out=bk_sb, in_=bkp[:, :])
            qeng[2].dma_start(out=bq_sb, in_=bqp[:, :])
            qeng[2].dma_start(out=bv_sb, in_=bvp[:, :])
            qeng[0].dma_start(out=poff_sb, in_=poffp[:, :])

            # Warm the PE HAM clock gate (~3.4us of activity flips it from
            # 1.2 to 2.4 GHz) with throwaway matmuls while the first input
            # DMAs are still in flight.
            warm_ps = psA.tile([P, P], f32, tag="warm", name="warm_ps", bufs=1)
            for _ in range(NWARM):
                nc.tensor.matmul(warm_ps, lhsT=ones, rhs=ones, start=True, stop=True)

            # K projection first (kc-major; x is fully resident by ~20us
            # thanks to the whole-chunk DMAs) so the KT half publishes ~14us
            # earlier and the whole collective chain (KT -> V -> s) lands
            # with slack instead of just-in-time.
            for kc in range(QC):
                for e in range(ET):
                    ps = psA.tile([P, NQ], f32, tag="ps")
                    for d in range(DT):
                        nc.tensor.matmul(
                            ps,
                            lhsT=wk_sb[:, d, e * P : (e + 1) * P],
                            rhs=x_sb[d][:, kc * NQ : (kc + 1) * NQ],
                            start=(d == 0),
                            stop=(d == DT - 1),
                        )
                    nc.scalar.activation(
                        out=kt_loc[:, e, kc * NQ : (kc + 1) * NQ],
                        in_=ps,
                        func=AF.Identity,
                        bias=bk_sb[:, e : e + 1],
                        scale=1.0,
                    )
            for e in range(ET):
                qeng[2 - (e % 2)].dma_start(
                    out=ktl_d[e * P : (e + 1) * P, :], in_=kt_loc[:, e, :]
                )
            # Start the KT exchange immediately: the CC engine is idle and
            # the partner half gates the partner-score blocks.
            pairs = [[2 * i, 2 * i + 1] for i in range(NCORES // 2)]
            nc.gpsimd.collective_compute(
                "AllGather",
                mybir.AluOpType.bypass,
                replica_groups=pairs,
                ins=[ktl_d.opt()],
                outs=[ktg_d.opt()],
            )
            # V local half (no bias; bv folded in at the end); the DVE
            # accumulates colsum(V_f32) for the rank-1 fp8 correction.
            vacc = outp.tile([P, D], f32, tag="vacc", bufs=1)
            for k in range(HKT):
                ps = psA.tile([P, D], f32, tag="ps")
                for d in range(DT):
                    nc.tensor.matmul(
                        ps,
                        lhsT=x_sb[d][:, k * P : (k + 1) * P],
                        rhs=wv_sb[:, d, :],
                        start=(d == 0),
                        stop=(d == DT - 1),
                    )
                nc.scalar.copy(out=v_loc[:, k, :], in_=ps)
                if k == 0:
                    nc.vector.tensor_copy(out=vacc, in_=ps)
                else:
                    nc.vector.tensor_add(vacc, vacc, ps)
                qeng[k % 3].dma_start(out=vl_d[k * P : (k + 1) * P, :], in_=v_loc[:, k, :])
            nc.gpsimd.collective_compute(
                "AllGather",
                mybir.AluOpType.bypass,
                replica_groups=pairs,
                ins=[vl_d.opt()],
                outs=[vg_d.opt()],
            )
            # s_local = colsum(V_f32) - colsum(V_fp8): partition-reduce vacc
            # with a ones-matmul; colsum the fp8 tiles with an all-ones fp8
            # DoubleRow matmul (exact f32 accumulation).
            vacc_bf = outp.tile([P, D], bf, tag="vacc_bf", bufs=1)
            nc.vector.tensor_copy(out=vacc_bf, in_=vacc)
            psc = psA.tile([1, D], f32, tag="c1", bufs=1)
            nc.tensor.matmul(psc, lhsT=ones[:, 0:1], rhs=vacc_bf, start=True, stop=True)
            psv = psA.tile([1, D], f32, tag="c2", bufs=1)
            for k in range(HKT):
                nc.tensor.matmul(
                    psv,
                    lhsT=ones_f8,
                    rhs=v_loc[:, k, :],
                    start=(k == 0),
                    stop=(k == HKT - 1),
                )
            sl_c = outp.tile([1, D], f32, tag="sl_c", bufs=1)
            nc.scalar.copy(out=sl_c, in_=psc)
            sl_sb = outp.tile([1, D], f32, tag="sl_sb", bufs=1)
            nc.vector.tensor_sub(sl_sb, sl_c, psv)
            nc.sync.dma_start(out=sl_d, in_=sl_sb)
            nc.gpsimd.collective_compute(
                "AllGather",
                mybir.AluOpType.bypass,
                replica_groups=pairs,
                ins=[sl_d.opt()],
                outs=[sg_d.opt()],
            )

            # Partner-half loads from the gather outputs, issued before the
            # Q projection so the transfers start the moment each gather
            # lands. The row base is rank-dependent, supplied by the host
            # via `poff` and applied as a dynamic (register) offset.
            SP = [mybir.EngineType.SP]
            kt_base = nc.values_load(
                poff_sb[0:1, 0:1], engines=SP,
                min_val=0, max_val=ET * P,
                skip_runtime_bounds_check=True,
            )
            nc.sync.dma_start(
                out=kt_rem,
                in_=ktg_d[bass.ds(kt_base, ET * P), :].rearrange(
                    "(e p) c -> p e c", p=P
                ),
            )
            v_base = nc.values_load(
                poff_sb[0:1, 1:2], engines=SP,
                min_val=0, max_val=HKT * P,
                skip_runtime_bounds_check=True,
            )
            nc.sync.dma_start(
                out=v_rem,
                in_=vg_d[bass.ds(v_base, HKT * P), :].rearrange(
                    "(j p) c -> p j c", p=P
                ),
            )
            s_a = outp.tile([P, ET], f32, tag="s_a", bufs=1)
            s_b = outp.tile([P, ET], f32, tag="s_b", bufs=1)
            nc.sync.dma_start(
                out=s_a, in_=sg_d[0:1, :].rearrange("r (et p) -> (r p) et", p=P)
            )
            nc.sync.dma_start(
                out=s_b, in_=sg_d[1:2, :].rearrange("r (et p) -> (r p) et", p=P)
            )

        # ---- Phase 3: attention ----
        with (
            tc.tile_pool(name="pt", bufs=1) as pt_pool,
            tc.tile_pool(name="ps_st", bufs=2, space="PSUM") as ps_st,
            tc.tile_pool(name="ps_ot", bufs=3, space="PSUM") as ps_ot,
            tc.tile_pool(name="ps_rs", bufs=1, space="PSUM") as ps_rs,
        ):
            ptl_tiles = {}
            ptp_tiles = {}
            recips = {}

            # Q projection: chunk 0 eagerly (ACT evacuation: the exp stream
            # hasn't started); chunks 1-3 interleaved into the local score
            # blocks with DVE evacuation.
            def q_mms(qc, e, d0, d1, ps):
                for d in range(d0, d1):
                    nc.tensor.matmul(
                        ps,
                        lhsT=wq_sb[:, d, e * P : (e + 1) * P],
                        rhs=x_sb[d][:, qc * NQ : (qc + 1) * NQ],
                        start=(d == 0),
                        stop=(d == DT - 1),
                    )

            for e in range(ET):
                ps = ps_ot.tile([P, NQ], f32, tag="ot", name=f"q0e{e}")
                q_mms(0, e, 0, DT, ps)
                nc.scalar.activation(
                    out=qt_sb[:, e, 0:NQ],
                    in_=ps,
                    func=AF.Identity,
                    bias=bq_sb[:, e : e + 1],
                    scale=1.0,
                )

            def pt_half(qc, half):
                return ptl_tiles[qc] if half == 0 else ptp_tiles[qc]

            q_ps = {}

            def slot(qc, half, j, av=None):
                """One k-tile-pair score slot: 4 DR matmuls -> exp -> rowsum
                (alternating DVE/GpSimd). `av` optionally appends AV or
                Q-projection matmuls to the PE stream inside this slot."""
                qsl = slice(qc * NQ, (qc + 1) * NQ)
                if j == 0:
                    t = pt_pool.tile(
                        [P, HKT, NQ], f8,
                        tag=("ptl" if half == 0 else "ptp"),
                        name=f"pt{half}_{qc}",
                        bufs=(4 if half == 0 else 3),
                    )
                    if half == 0:
                        ptl_tiles[qc] = t
                    else:
                        ptp_tiles[qc] = t
                ps = ps_st.tile([P, 2, NQ], f32, tag="st", name="st_ps")
                for kh in range(2):
                    kk = 2 * j + kh
                    for e in (0, 2):
                        if half == 0:
                            lhsT = kt_loc[:, e : e + 2, kk * P : (kk + 1) * P]
                        else:
                            lhsT = kt_rem[:, e : e + 2, kk * P : (kk + 1) * P]
                        nc.tensor.matmul(
                            ps[:, kh, :],
                            lhsT=lhsT,
                            rhs=qt_sb[:, e : e + 2, qsl],
                            start=(e == 0),
                            stop=(e == 2),
                            perf_mode=DR,
                        )
                pt_pair = pt_half(qc, half)[:, 2 * j : 2 * j + 2, :]
                nc.scalar.activation(out=pt_pair, in_=ps, func=AF.Exp, scale=SCALE)
                # Interleaved work (AV, Q-projection, rowsum matmuls) is
                # emitted after the exp so the ACT queue stays pure exps.
                if av is not None:
                    av(j)
                if half == 1:
                    rs_slot(qc, j)

            rs_tiles = {}

            def rs_mm(qc, p, start, stop):
                # One all-ones fp8 DoubleRow rowsum matmul (PT pair p),
                # accumulated into the chunk's PSUM bank; the result is
                # partition-replicated.
                if p < NPAIR:
                    prhs = ptl_tiles[qc][:, 2 * p : 2 * p + 2, :]
                else:
                    prhs = ptp_tiles[qc][:, 2 * (p - NPAIR) : 2 * (p - NPAIR) + 2, :]
                nc.tensor.matmul(
                    rs_tiles[qc],
                    lhsT=ones_f8w,
                    rhs=prhs,
                    start=start,
                    stop=stop,
                    perf_mode=DR,
                )

            def rs_slot(qc, j):
                # Rowsum matmuls ride inside the partner block P(qc): local
                # pairs (ready since L(qc)) in slots 0-3, remote pair p two
                # slots behind its exp. Only remote pairs 6-7 and the
                # reciprocal remain at the block boundary, so the first AV
                # evacuation of the next block never waits on the chain.
                if j == 0:
                    rs_tiles[qc] = ps_rs.tile([P, NQ], f32, tag="rs", name=f"rs{qc}")
                if j < 4:
                    rs_mm(qc, 2 * j, start=(j == 0), stop=False)
                    rs_mm(qc, 2 * j + 1, start=False, stop=False)
                if j >= 2:
                    rs_mm(qc, NPAIR + (j - 2), start=False, stop=False)

            def rowsum_finish(qc):
                rs_mm(qc, 2 * NPAIR - 2, start=False, stop=False)
                rs_mm(qc, 2 * NPAIR - 1, start=False, stop=True)
                recips[qc] = outp.tile(
                    [P, NQ], f32, tag="recip", bufs=2, name=f"recip{qc}"
                )
                nc.vector.reciprocal(recips[qc], rs_tiles[qc])

            def bias2_compute():
                # bias2 = bv + (s_local + s_partner) / S on the (idle)
                # GpSimd engine: these [128, 4] ops are tiny even at Pool
                # throughput, and their wait on the s AllGather (~110us)
                # cannot back up the DVE mult/reciprocal stream.
                nc.gpsimd.tensor_add(s_a, s_a, s_b)
                for et in range(ET):
                    nc.gpsimd.tensor_scalar(
                        out=bias2[:, et : et + 1],
                        in0=s_a[:, et : et + 1],
                        scalar1=1.0 / S,
                        scalar2=bv_sb[:, et : et + 1],
                        op0=ALU.mult,
                        op1=ALU.add,
                    )

            av_state = {}

            def av_evac(qc, e, ops, halves=1):
                # Normalize on the DVE, bias-add on ACT (emitted after the
                # slot's exp, so it never sits in front of pending exps; the
                # ACT stream has ~10us of slack per AV-carrying block).
                # `halves=2` pipelines the mult/bias/DMA chain in 256-col
                # pieces — used on the final chunk to shorten the tail.
                tmp = outp.tile([P, NQ], f32, tag="tmp", bufs=3)
                hw = NQ // halves
                for hh in range(halves):
                    hs = slice(hh * hw, (hh + 1) * hw)
                    qsl = slice(qc * NQ + hh * hw, qc * NQ + (hh + 1) * hw)
                    nc.vector.tensor_mul(tmp[:, hs], ops[:, hs], recips[qc][:, hs])
                    nc.scalar.activation(
                        out=tmp[:, hs],
                        in_=tmp[:, hs],
                        func=AF.Identity,
                        bias=bias2[:, e : e + 1],
                        scale=1.0,
                    )
                    (nc.sync if (e + hh) % 2 == 0 else nc.gpsimd).dma_start(
                        out=ot[e * P : (e + 1) * P, qsl], in_=tmp[:, hs]
                    )

            def av_mms(qc, e, ops, p0, p1):
                for p in range(p0, p1):
                    if p < NPAIR:
                        vlhsT = v_loc[:, 2 * p : 2 * p + 2, e * P : (e + 1) * P]
                        prhs = ptl_tiles[qc][:, 2 * p : 2 * p + 2, :]
                    else:
                        pp = p - NPAIR
                        vlhsT = v_rem[:, 2 * pp : 2 * pp + 2, e * P : (e + 1) * P]
                        prhs = ptp_tiles[qc][:, 2 * pp : 2 * pp + 2, :]
                    nc.tensor.matmul(
                        ops,
                        lhsT=vlhsT,
                        rhs=prhs,
                        start=(p == 0),
                        stop=(p == 2 * NPAIR - 1),
                        perf_mode=DR,
                    )

            def av_slot(qc):
                # 8 AV matmuls per score slot, e-major: e-group e occupies
                # slots 2e (pairs 0-7) and 2e+1 (pairs 8-15 + evacuation).
                def fn(j):
                    e = j // 2
                    if j % 2 == 0:
                        av_state[qc] = ps_ot.tile(
                            [P, NQ], f32, tag="ot", name=f"av{qc}e{e}"
                        )
                        av_mms(qc, e, av_state[qc], 0, NPAIR)
                    else:
                        av_mms(qc, e, av_state[qc], NPAIR, 2 * NPAIR)
                        av_evac(qc, e, av_state[qc], halves=(2 if qc == QC - 1 else 1))
                return fn

            def q_slot(qc):
                # Q-projection for chunk qc rides in the previous local
                # score block, 2 matmuls per slot (e = j//2), with DVE
                # evacuation so the ACT exp stream is untouched.
                def fn(j):
                    e = j // 2
                    if j % 2 == 0:
                        q_ps[qc] = ps_ot.tile([P, NQ], f32, tag="ot", name=f"q{qc}e{e}")
                        q_mms(qc, e, 0, 2, q_ps[qc])
                    else:
                        q_mms(qc, e, 2, DT, q_ps[qc])
                        nc.vector.tensor_scalar(
                            out=qt_sb[:, e, qc * NQ : (qc + 1) * NQ],
                            in0=q_ps[qc],
                            scalar1=bq_sb[:, e : e + 1],
                            scalar2=None,
                            op0=ALU.add,
                        )
                return fn

            # Local-half score blocks, with Q chunks 1-3 interleaved.
            for qc in range(QC):
                av = q_slot(qc + 1) if qc < QC - 1 else None
                for j in range(NPAIR):
                    slot(qc, 0, j, av=av)
            # Partner-half blocks: P0 bare; av(qc) rides inside P(qc+1);
            # each chunk's rowsum matmul burst follows its last score block.
            for j in range(NPAIR):
                slot(0, 1, j)
            rowsum_finish(0)
            bias2_compute()
            for qc in range(1, QC):
                avfn = av_slot(qc - 1)
                for j in range(NPAIR):
                    slot(qc, 1, j, av=avfn)
                rowsum_finish(qc)
            # Chunk 3's AV tail runs after the last score block.
            avfn = av_slot(QC - 1)
            for j in range(2 * ET):
                avfn(j)

    _split_excess_waits(nc, mybir)
    return nc


def _get_nc():
    if "nc" not in _CACHE:
        _CACHE["nc"] = _build_nc()
    return _CACHE["nc"]


def _make_in_maps(x, Wq, bq, Wk, bk, Wv, bv):
    bf16 = ml_dtypes.bfloat16
    def _retile(W):
        # [P, DT*D] with w_re[p, d*D+j] = W.T[d*P+p, j]: per-partition
        # contiguous 4KB runs so the whole matrix loads as one DMA.
        return np.ascontiguousarray(
            W.T.reshape(DT, P, D).transpose(1, 0, 2).reshape(P, DT * D)
        ).astype(bf16)

    wqT = _retile(Wq)
    wkT = _retile(Wk)
    wvT = _retile(Wv)
    bqp = np.ascontiguousarray(bq.reshape(ET, P).T).astype(np.float32)
    bkp = np.ascontiguousarray(bk.reshape(ET, P).T).astype(np.float32)
    bvp = np.ascontiguousarray(bv.reshape(ET, P).T).astype(np.float32)
    in_maps = []
    for c in range(NCORES):
        b, h = divmod(c, 2)
        # Local half of x[b].T: both this core's query columns and its K/V
        # half (they are the same row range by construction).
        xTl = np.ascontiguousarray(x[b, h * SQ : (h + 1) * SQ, :].T).astype(bf16)
        # Partner-half row bases into the rank-ordered AllGather outputs.
        poff = np.array(
            [[(1 - h) * ET * P, (1 - h) * HKT * P]], dtype=np.uint32
        )
        in_maps.append(
            {
                "xT": xTl,
                "poff": poff,
                "wqT": wqT,
                "wkT": wkT,
                "wvT": wvT,
                "bq": bqp,
                "bk": bkp,
                "bv": bvp,
            }
        )
    return in_maps


def _run(in_maps, **kwargs):
    from concourse.bass_utils import run_bass_kernel_spmd

    nc = _get_nc()
    return run_bass_kernel_spmd(nc, in_maps, core_ids=list(range(NCORES)), **kwargs)


def kernel(x, Wq, bq, Wk, bk, Wv, bv):
    x = np.asarray(x, dtype=np.float32)
    Wq = np.asarray(Wq, dtype=np.float32)
    Wk = np.asarray(Wk, dtype=np.float32)
    Wv = np.asarray(Wv, dtype=np.float32)
    bq = np.asarray(bq, dtype=np.float32)
    bk = np.asarray(bk, dtype=np.float32)
    bv = np.asarray(bv, dtype=np.float32)

    res = _run(_make_in_maps(x, Wq, bq, Wk, bk, Wv, bv))
    out = np.empty((B, S, D), dtype=np.float32)
    for c in range(NCORES):
        b, h = divmod(c, 2)
        out[b, h * SQ : (h + 1) * SQ, :] = np.asarray(res.results[c]["ot"]).T
    return out


# revision 28
# speedup vs baseline: 1.0243x; 1.0243x over previous
"""BertSelfAttention (B=4, S=4096, D=512) on 8 TRN2 NeuronCores.

Sharding: core c handles batch b = c//2 and query-row half h = c%2
(2048 q rows). K/V are computed on-core for the full 4096 keys of that
batch (halves exchanged within each core pair), avoiding big collectives.

Layout trick: everything is computed transposed so no on-device
transposes are needed:
  QT[e, q] = Wq @ x.T          (lhsT = WqT chunks, rhs = xT chunks)
  KT[e, k] = Wk @ x.T
  V [k, e] = x @ Wv.T          (lhsT = xT chunks,  rhs = WvT)
  ST[k, q] = K Q.T             (lhsT = KT chunks,  rhs = QT)   -> exp -> PT
  OT[e, q] = V.T P.T           (lhsT = V chunks,   rhs = PT)
Softmax runs without max-subtraction (scores are ~N(0, 0.3^2), so exp
cannot overflow and the result is mathematically identical).

Precision: projections run in bf16 (fp32 PSUM); Q/K/V/P are quantized
to fp8e4 (e4m3) and the two big matmuls run as fp8 DoubleRow matmuls
(256-deep contraction per pass = 2x bf16 MAC rate on HW). The rank-1
component of the V-quantization error (softmax rows sum to one) is
corrected by adding (colsum(V_f32) - colsum(V_fp8))/S to the output
bias; colsum(V_f32) accumulates on the DVE during the V projection and
colsum(V_fp8) is an all-ones fp8 matmul, so the correction costs ~2us.

Schedule (trace-driven rewrite of the 209.6us baseline; ~205.7us):
- v1 put the softmax row-sums on the DVE (2 adds/pair): the DVE fell
  ~9us behind the exp stream, and its rowsum -> cast -> ones-matmul ->
  3.4us-reciprocal chain stalled the PE twice (8.9us + 4.3us) with
  clock-droop penalties on each restart. Here the row-sums are 16
  all-ones fp8 DoubleRow matmuls per chunk (+216ns/pair on the PE,
  partition-replicated result in one PSUM bank), emitted right after
  the chunk's partner score block; the reciprocal follows immediately
  and the DVE stays ~60% idle. (GpSimd tensor ops measured 3-15x
  slower than the cost model — eff ~0.05 for tensor_scalar — so
  offloading element work to Pool is a dead end on this HW.)
- AV matmuls for q-chunk qc are interleaved per-pair into the NEXT
  chunk's partner-half score block (8 av matmuls per score slot,
  e-major), so the PE runs back-to-back instead of alternating
  exp-paced score stretches with pure-AV stretches. Evacuation:
  DVE normalize-multiply, ACT bias-add (emitted after the slot's exp
  so the ACT queue stays exps-in-order), DMA out on alternating
  queues; the final chunk evacuates in 256-col halves to shorten the
  drain tail.
- K and V projections are fused per x column-chunk (4 KT e-tiles + 4 V
  k-tiles per chunk, ~7us of PE work per 512KB of arriving x), so the
  PE rides just behind the input DMA stream instead of stalling on it.
  Q chunks 1-3 ride inside the local score blocks (2 matmuls/slot,
  DVE evacuation); only K, V and Q-chunk-0 gate attention start.
- Inputs load via the 3 DMA-capable queues (SP/GpSimd/ACT): wk+wv
  first, then x column-chunks striped in consumption order; ~96
  throwaway matmuls warm the PE clock gate during the DMA wait.
- Collective order matters: KT, then V, then the tiny s exchange —
  inserting s before V costs a full collective-startup latency on the
  1MB V gather and starves the AV phase (measured +37us).
- PSUM: 2x2-bank score tiles, a 3-buffer [128,512] pool shared by AV
  accumulators and Q-projection tiles, and 1 rowsum bank (8/8 banks).
"""

import sys

for _p in ("/opt/trn_rl_repo", "/root/.axon_site/_ro/trn_rl_repo"):
    if _p not in sys.path:
        sys.path.append(_p)

import numpy as np
import ml_dtypes

B, S, D = 4, 4096, 512
NCORES = 8
SQ = S // 2  # query rows per core
P = 128
NQ = 512  # q-chunk width (moving free dim)
DT = D // P  # 4 contraction chunks for d
ET = D // P  # 4 e tiles
KTI = S // P  # 32 k tiles
QC = SQ // NQ  # 4 q chunks per core
HKT = KTI // 2  # 16 local k-tiles per core
HS = S // 2  # 2048 local keys per core
NPAIR = HKT // 2  # 8 k-tile pairs per half-block
SCALE = 1.0 / float(np.sqrt(np.float32(D)))
NWARM = 96

_CACHE = {}


def _split_excess_waits(nc, mybir, max_waits=1):
    """This walrus build rejects instructions carrying more than a couple of
    sync waits. Cap every instruction at `max_waits`, spilling the rest onto
    same-engine InstNoOps inserted immediately before it (equivalent
    semantics: the engine's stream stalls at the nop instead)."""
    for f in nc.m.functions:
        for bb in f.blocks:
            old = list(bb.instructions)
            new = []
            for inst in old:
                si = inst.sync_info
                waits = list(si.on_wait) if si is not None and si.on_wait else []
                if len(waits) > max_waits:
                    keep = waits[-max_waits:]
                    excess = waits[:-max_waits]
                    for i in range(0, len(excess), max_waits):
                        nop = mybir.InstNoOp(
                            name=f"waitnop-{nc.next_id()}", ins=[], outs=[]
                        )
                        nop.engine = inst.engine
                        nop.sync_info = mybir.SyncInfo(
                            on_wait=excess[i : i + max_waits], on_update=[]
                        )
                        new.append(nop)
                    inst.sync_info = mybir.SyncInfo(
                        on_wait=keep,
                        on_update=list(si.on_update) if si.on_update else [],
                    )
                new.append(inst)
            if len(new) != len(old):
                bb.instructions[:] = new


def _build_nc():
    import concourse.bass as bass
    import concourse.mybir as mybir
    import concourse.tile as tile
    from contextlib import ExitStack

    bf = mybir.dt.bfloat16
    f32 = mybir.dt.float32
    f8 = mybir.dt.float8e4
    AF = mybir.ActivationFunctionType
    DR = mybir.MatmulPerfMode.DoubleRow
    ALU = mybir.AluOpType

    u32 = mybir.dt.uint32
    nc = bass.Bass()
    xT = nc.declare_dram_parameter("xT", [D, SQ], bf, isOutput=False)
    # Weights host-retiled to [P, DT*D]: w_re[p, d*D+j] = wT[d*P+p, j], so
    # each partition's data is one contiguous 4KB run and the whole matrix
    # loads as a single high-throughput DMA.
    wqT = nc.declare_dram_parameter("wqT", [P, DT * D], bf, isOutput=False)
    wkT = nc.declare_dram_parameter("wkT", [P, DT * D], bf, isOutput=False)
    wvT = nc.declare_dram_parameter("wvT", [P, DT * D], bf, isOutput=False)
    bqp = nc.declare_dram_parameter("bq", [P, ET], f32, isOutput=False)
    bkp = nc.declare_dram_parameter("bk", [P, ET], f32, isOutput=False)
    bvp = nc.declare_dram_parameter("bv", [P, ET], f32, isOutput=False)
    # Host-computed row bases into the AllGather outputs for the PARTNER
    # half (rank-dependent: (1-h)*512 + e*128 for KT, (1-h)*2048 + j*128
    # for V). Drives dynamic (register-offset) DMAs.
    poffp = nc.declare_dram_parameter("poff", [1, 2], u32, isOutput=False)
    ot = nc.declare_dram_parameter("ot", [D, SQ], f32, isOutput=True)

    with tile.TileContext(nc) as tc, ExitStack() as ctx:
        const_pool = ctx.enter_context(tc.tile_pool(name="const", bufs=1))
        persist = ctx.enter_context(tc.tile_pool(name="persist", bufs=1))
        outp = ctx.enter_context(tc.tile_pool(name="outp", bufs=2))
        xin_pool = ctx.enter_context(tc.tile_pool(name="xin", bufs=1))

        ones = const_pool.tile([P, P], bf, tag="ones")
        nc.vector.memset(ones, 1.0)
        ones_f8 = const_pool.tile([P, 1], f8, tag="ones8")
        nc.gpsimd.memset(ones_f8, 1.0)
        # Wide all-ones fp8 stationary for the DoubleRow rowsum matmuls
        # (partition-reduces a PT pair and replicates across partitions).
        ones_f8w = const_pool.tile([P, 2, P], f8, tag="ones8w")
        nc.gpsimd.memset(ones_f8w, 1.0)
        bq_sb = const_pool.tile([P, ET], f32, tag="bq")
        bk_sb = const_pool.tile([P, ET], f32, tag="bk")
        bv_sb = const_pool.tile([P, ET], f32, tag="bv")
        wq_sb = const_pool.tile([P, DT, D], bf, tag="wq", name="wq")
        wk_sb = const_pool.tile([P, DT, D], bf, tag="wk", name="wk")
        wv_sb = const_pool.tile([P, DT, D], bf, tag="wv", name="wv")
        # fp8 operand tiles for the DoubleRow matmuls; contraction-paired
        # chunks live in dim 1 so [:, e:e+2, cols] is a valid 3D AP.
        qt_sb = persist.tile([P, ET, SQ], f8, tag="qt", name="qt")
        # K/V k-order per core: [my half, partner half]. Separate tiles per
        # half so partner DMA-writes create no false deps on local reads.
        kt_loc = persist.tile([P, ET, HS], f8, tag="ktl", name="ktl")
        kt_rem = persist.tile([P, ET, HS], f8, tag="ktr", name="ktr")
        v_loc = persist.tile([P, HKT, D], f8, tag="vl", name="vl")
        v_rem = persist.tile([P, HKT, D], f8, tag="vr", name="vr")
        poff_sb = const_pool.tile([1, 2], u32, tag="poff")
        bias2 = const_pool.tile([P, ET], f32, tag="bias2")

        # ---- Phase 1+2: load inputs on 4 DMA queues, project K + local V,
        # AllGather the K/V halves within each core pair, project Q chunk 0.
        with (
            tc.tile_pool(name="psA", bufs=4, space="PSUM") as psA,
            tc.tile_pool(name="dram", bufs=1, space="DRAM") as dram,
        ):
            ktl_d = dram.tile([ET * P, HS], f8, tag="ktl_d")
            ktg_d = dram.tile([2 * ET * P, HS], f8, tag="ktg_d")
            vl_d = dram.tile([HKT * P, D], f8, tag="vl_d")
            vg_d = dram.tile([2 * HKT * P, D], f8, tag="vg_d")
            sl_d = dram.tile([1, D], f32, tag="sl_d")
            sg_d = dram.tile([2, D], f32, tag="sg_d")

            x_sb = [xin_pool.tile([P, HS], bf, tag=f"x{d}", name=f"x{d}") for d in range(DT)]
            # 3 parallel hardware DMA queues (SP, GpSimd, ACT — the only
            # DMA-capable engines); per queue: wk chunks first (the first
            # projection needs them), then x column-chunks in first-consumer
            # order, then the later-needed weights/biases.
            qeng = [nc.sync, nc.gpsimd, nc.scalar]
            # Whole-tensor transfers with 4KB-contiguous runs per partition:
            # wk + wv upfront (the fused K/V projection needs both), the
            # four x d-chunks (full 2048-col rows), then the late-needed wq.
            qeng[0].dma_start(
                out=wk_sb, in_=wkT[:, :].rearrange("p (d c) -> p d c", d=DT)
            )
            qeng[1].dma_start(
                out=wv_sb, in_=wvT[:, :].rearrange("p (d c) -> p d c", d=DT)
            )
            for d in range(DT):
                qeng[(2 + d) % 3].dma_start(
                    out=x_sb[d], in_=xT[d * P : (d + 1) * P, :]
                )
            qeng[1].dma_start(
                out=wq_sb, in_=wqT[:, :].rearrange("p (d c) -> p d c", d=DT)
            )
            qeng[1].dma_start(out=bk_sb, in_=bkp[:, :])
            qeng[2].dma_start(out=bq_sb, in_=bqp[:, :])
            qeng[2].dma_start(out=bv_sb, in_=bvp[:, :])
            qeng[0].dma_start(out=poff_sb, in_=poffp[:, :])

            # Warm the PE HAM clock gate (~3.4us of activity flips it from
            # 1.2 to 2.4 GHz) with throwaway matmuls while the first input
            # DMAs are still in flight.
            warm_ps = psA.tile([P, P], f32, tag="warm", name="warm_ps", bufs=1)
            for _ in range(NWARM):
                nc.tensor.matmul(warm_ps, lhsT=ones, rhs=ones, start=True, stop=True)

            # Fused K+V projection, one x column-chunk at a time: each
            # arriving 512-col x chunk unlocks ~7us of PE work (4 KT e-tiles
            # + 4 V k-tiles), so the PE never waits on the x DMA stream.
            # K bias fused on ACT evacuation; V copied to fp8 on ACT with
            # the DVE accumulating colsum(V_f32) for the rank-1 correction.
            vacc = outp.tile([P, D], f32, tag="vacc", bufs=1)
            for kc in range(QC):
                for e in range(ET):
                    ps = psA.tile([P, NQ], f32, tag="ps")
                    for d in range(DT):
                        nc.tensor.matmul(
                            ps,
                            lhsT=wk_sb[:, d, e * P : (e + 1) * P],
                            rhs=x_sb[d][:, kc * NQ : (kc + 1) * NQ],
                            start=(d == 0),
                            stop=(d == DT - 1),
                        )
                    nc.scalar.activation(
                        out=kt_loc[:, e, kc * NQ : (kc + 1) * NQ],
                        in_=ps,
                        func=AF.Identity,
                        bias=bk_sb[:, e : e + 1],
                        scale=1.0,
                    )
                for kk in range(4):
                    k = 4 * kc + kk
                    ps = psA.tile([P, D], f32, tag="ps")
                    for d in range(DT):
                        nc.tensor.matmul(
                            ps,
                            lhsT=x_sb[d][:, k * P : (k + 1) * P],
                            rhs=wv_sb[:, d, :],
                            start=(d == 0),
                            stop=(d == DT - 1),
                        )
                    nc.scalar.copy(out=v_loc[:, k, :], in_=ps)
                    if k == 0:
                        nc.vector.tensor_copy(out=vacc, in_=ps)
                    else:
                        nc.vector.tensor_add(vacc, vacc, ps)
                    qeng[k % 3].dma_start(out=vl_d[k * P : (k + 1) * P, :], in_=v_loc[:, k, :])
            for e in range(ET):
                qeng[2 - (e % 2)].dma_start(
                    out=ktl_d[e * P : (e + 1) * P, :], in_=kt_loc[:, e, :]
                )
            # Start the KT exchange immediately: the CC engine is idle and
            # the partner half gates the partner-score blocks.
            pairs = [[2 * i, 2 * i + 1] for i in range(NCORES // 2)]
            nc.gpsimd.collective_compute(
                "AllGather",
                mybir.AluOpType.bypass,
                replica_groups=pairs,
                ins=[ktl_d.opt()],
                outs=[ktg_d.opt()],
            )
            nc.gpsimd.collective_compute(
                "AllGather",
                mybir.AluOpType.bypass,
                replica_groups=pairs,
                ins=[vl_d.opt()],
                outs=[vg_d.opt()],
            )
            # s_local = colsum(V_f32) - colsum(V_fp8): partition-reduce vacc
            # with a ones-matmul; colsum the fp8 tiles with an all-ones fp8
            # DoubleRow matmul (exact f32 accumulation).
            vacc_bf = outp.tile([P, D], bf, tag="vacc_bf", bufs=1)
            nc.vector.tensor_copy(out=vacc_bf, in_=vacc)
            psc = psA.tile([1, D], f32, tag="c1", bufs=1)
            nc.tensor.matmul(psc, lhsT=ones[:, 0:1], rhs=vacc_bf, start=True, stop=True)
            psv = psA.tile([1, D], f32, tag="c2", bufs=1)
            for k in range(HKT):
                nc.tensor.matmul(
                    psv,
                    lhsT=ones_f8,
                    rhs=v_loc[:, k, :],
                    start=(k == 0),
                    stop=(k == HKT - 1),
                )
            sl_c = outp.tile([1, D], f32, tag="sl_c", bufs=1)
            nc.scalar.copy(out=sl_c, in_=psc)
            sl_sb = outp.tile([1, D], f32, tag="sl_sb", bufs=1)
            nc.vector.tensor_sub(sl_sb, sl_c, psv)
            nc.sync.dma_start(out=sl_d, in_=sl_sb)
            nc.gpsimd.collective_compute(
                "AllGather",
                mybir.AluOpType.bypass,
                replica_groups=pairs,
                ins=[sl_d.opt()],
                outs=[sg_d.opt()],
            )

            # Partner-half loads from the gather outputs, issued before the
            # Q projection so the transfers start the moment each gather
            # lands. The row base is rank-dependent, supplied by the host
            # via `poff` and applied as a dynamic (register) offset.
            SP = [mybir.EngineType.SP]
            kt_base = nc.values_load(
                poff_sb[0:1, 0:1], engines=SP,
                min_val=0, max_val=ET * P,
                skip_runtime_bounds_check=True,
            )
            nc.sync.dma_start(
                out=kt_rem,
                in_=ktg_d[bass.ds(kt_base, ET * P), :].rearrange(
                    "(e p) c -> p e c", p=P
                ),
            )
            v_base = nc.values_load(
                poff_sb[0:1, 1:2], engines=SP,
                min_val=0, max_val=HKT * P,
                skip_runtime_bounds_check=True,
            )
            nc.sync.dma_start(
                out=v_rem,
                in_=vg_d[bass.ds(v_base, HKT * P), :].rearrange(
                    "(j p) c -> p j c", p=P
                ),
            )
            s_a = outp.tile([P, ET], f32, tag="s_a", bufs=1)
            s_b = outp.tile([P, ET], f32, tag="s_b", bufs=1)
            nc.sync.dma_start(
                out=s_a, in_=sg_d[0:1, :].rearrange("r (et p) -> (r p) et", p=P)
            )
            nc.sync.dma_start(
                out=s_b, in_=sg_d[1:2, :].rearrange("r (et p) -> (r p) et", p=P)
            )

        # ---- Phase 3: attention ----
        with (
            tc.tile_pool(name="pt", bufs=1) as pt_pool,
            tc.tile_pool(name="ps_st", bufs=2, space="PSUM") as ps_st,
            tc.tile_pool(name="ps_ot", bufs=3, space="PSUM") as ps_ot,
            tc.tile_pool(name="ps_rs", bufs=1, space="PSUM") as ps_rs,
        ):
            ptl_tiles = {}
            ptp_tiles = {}
            recips = {}

            # Q projection: chunk 0 eagerly (ACT evacuation: the exp stream
            # hasn't started); chunks 1-3 interleaved into the local score
            # blocks with DVE evacuation.
            def q_mms(qc, e, d0, d1, ps):
                for d in range(d0, d1):
                    nc.tensor.matmul(
                        ps,
                        lhsT=wq_sb[:, d, e * P : (e + 1) * P],
                        rhs=x_sb[d][:, qc * NQ : (qc + 1) * NQ],
                        start=(d == 0),
                        stop=(d == DT - 1),
                    )

            for e in range(ET):
                ps = ps_ot.tile([P, NQ], f32, tag="ot", name=f"q0e{e}")
                q_mms(0, e, 0, DT, ps)
                nc.scalar.activation(
                    out=qt_sb[:, e, 0:NQ],
                    in_=ps,
                    func=AF.Identity,
                    bias=bq_sb[:, e : e + 1],
                    scale=1.0,
                )

            def pt_half(qc, half):
                return ptl_tiles[qc] if half == 0 else ptp_tiles[qc]

            q_ps = {}

            def slot(qc, half, j, av=None):
                """One k-tile-pair score slot: 4 DR matmuls -> exp -> rowsum
                (alternating DVE/GpSimd). `av` optionally appends AV or
                Q-projection matmuls to the PE stream inside this slot."""
                qsl = slice(qc * NQ, (qc + 1) * NQ)
                if j == 0:
                    t = pt_pool.tile(
                        [P, HKT, NQ], f8,
                        tag=("ptl" if half == 0 else "ptp"),
                        name=f"pt{half}_{qc}",
                        bufs=(4 if half == 0 else 3),
                    )
                    if half == 0:
                        ptl_tiles[qc] = t
                    else:
                        ptp_tiles[qc] = t
                ps = ps_st.tile([P, 2, NQ], f32, tag="st", name="st_ps")
                for kh in range(2):
                    kk = 2 * j + kh
                    for e in (0, 2):
                        if half == 0:
                            lhsT = kt_loc[:, e : e + 2, kk * P : (kk + 1) * P]
                        else:
                            lhsT = kt_rem[:, e : e + 2, kk * P : (kk + 1) * P]
                        nc.tensor.matmul(
                            ps[:, kh, :],
                            lhsT=lhsT,
                            rhs=qt_sb[:, e : e + 2, qsl],
                            start=(e == 0),
                            stop=(e == 2),
                            perf_mode=DR,
                        )
                pt_pair = pt_half(qc, half)[:, 2 * j : 2 * j + 2, :]
                nc.scalar.activation(out=pt_pair, in_=ps, func=AF.Exp, scale=SCALE)
                # Interleaved work (AV or Q-projection matmuls) is emitted
                # after the exp so the ACT queue stays a pure exp stream.
                if av is not None:
                    av(j)

            def rowsum(qc):
                # Rowsums on the PE: 16 all-ones fp8 DoubleRow matmuls over
                # the chunk's PT pairs, accumulated in one PSUM bank (the
                # result is partition-replicated), then the reciprocal.
                # Emitted right after P(qc): the last pair's exp completes
                # ~2us into the 3.5us matmul burst, so the PE barely waits.
                rs_ps = ps_rs.tile([P, NQ], f32, tag="rs", name=f"rs{qc}")
                for p in range(2 * NPAIR):
                    if p < NPAIR:
                        prhs = ptl_tiles[qc][:, 2 * p : 2 * p + 2, :]
                    else:
                        prhs = ptp_tiles[qc][:, 2 * (p - NPAIR) : 2 * (p - NPAIR) + 2, :]
                    nc.tensor.matmul(
                        rs_ps,
                        lhsT=ones_f8w,
                        rhs=prhs,
                        start=(p == 0),
                        stop=(p == 2 * NPAIR - 1),
                        perf_mode=DR,
                    )
                recips[qc] = outp.tile(
                    [P, NQ], f32, tag="recip", bufs=2, name=f"recip{qc}"
                )
                nc.vector.reciprocal(recips[qc], rs_ps)

            def bias2_compute():
                # bias2 = bv + (s_local + s_partner) / S on the (idle)
                # GpSimd engine: these [128, 4] ops are tiny even at Pool
                # throughput, and their wait on the s AllGather (~110us)
                # cannot back up the DVE mult/reciprocal stream.
                nc.gpsimd.tensor_add(s_a, s_a, s_b)
                for et in range(ET):
                    nc.gpsimd.tensor_scalar(
                        out=bias2[:, et : et + 1],
                        in0=s_a[:, et : et + 1],
                        scalar1=1.0 / S,
                        scalar2=bv_sb[:, et : et + 1],
                        op0=ALU.mult,
                        op1=ALU.add,
                    )

            av_state = {}

            def av_evac(qc, e, ops, halves=1):
                # Normalize on the DVE, bias-add on ACT (emitted after the
                # slot's exp, so it never sits in front of pending exps; the
                # ACT stream has ~10us of slack per AV-carrying block).
                # `halves=2` pipelines the mult/bias/DMA chain in 256-col
                # pieces — used on the final chunk to shorten the tail.
                tmp = outp.tile([P, NQ], f32, tag="tmp", bufs=3)
                hw = NQ // halves
                for hh in range(halves):
                    hs = slice(hh * hw, (hh + 1) * hw)
                    qsl = slice(qc * NQ + hh * hw, qc * NQ + (hh + 1) * hw)
                    nc.vector.tensor_mul(tmp[:, hs], ops[:, hs], recips[qc][:, hs])
                    nc.scalar.activation(
                        out=tmp[:, hs],
                        in_=tmp[:, hs],
                        func=AF.Identity,
                        bias=bias2[:, e : e + 1],
                        scale=1.0,
                    )
                    (nc.sync if (e + hh) % 2 == 0 else nc.gpsimd).dma_start(
                        out=ot[e * P : (e + 1) * P, qsl], in_=tmp[:, hs]
                    )

            def av_mms(qc, e, ops, p0, p1):
                for p in range(p0, p1):
                    if p < NPAIR:
                        vlhsT = v_loc[:, 2 * p : 2 * p + 2, e * P : (e + 1) * P]
                        prhs = ptl_tiles[qc][:, 2 * p : 2 * p + 2, :]
                    else:
                        pp = p - NPAIR
                        vlhsT = v_rem[:, 2 * pp : 2 * pp + 2, e * P : (e + 1) * P]
                        prhs = ptp_tiles[qc][:, 2 * pp : 2 * pp + 2, :]
                    nc.tensor.matmul(
                        ops,
                        lhsT=vlhsT,
                        rhs=prhs,
                        start=(p == 0),
                        stop=(p == 2 * NPAIR - 1),
                        perf_mode=DR,
                    )

            def av_slot(qc):
                # 8 AV matmuls per score slot, e-major: e-group e occupies
                # slots 2e (pairs 0-7) and 2e+1 (pairs 8-15 + evacuation).
                def fn(j):
                    e = j // 2
                    if j % 2 == 0:
                        av_state[qc] = ps_ot.tile(
                            [P, NQ], f32, tag="ot", name=f"av{qc}e{e}"
                        )
                        av_mms(qc, e, av_state[qc], 0, NPAIR)
                    else:
                        av_mms(qc, e, av_state[qc], NPAIR, 2 * NPAIR)
                        av_evac(qc, e, av_state[qc], halves=(2 if qc == QC - 1 else 1))
                return fn

            def q_slot(qc):
                # Q-projection for chunk qc rides in the previous local
                # score block, 2 matmuls per slot (e = j//2), with DVE
                # evacuation so the ACT exp stream is untouched.
                def fn(j):
                    e = j // 2
                    if j % 2 == 0:
                        q_ps[qc] = ps_ot.tile([P, NQ], f32, tag="ot", name=f"q{qc}e{e}")
                        q_mms(qc, e, 0, 2, q_ps[qc])
                    else:
                        q_mms(qc, e, 2, DT, q_ps[qc])
                        nc.vector.tensor_scalar(
                            out=qt_sb[:, e, qc * NQ : (qc + 1) * NQ],
                            in0=q_ps[qc],
                            scalar1=bq_sb[:, e : e + 1],
                            scalar2=None,
                            op0=ALU.add,
                        )
                return fn

            # Local-half score blocks, with Q chunks 1-3 interleaved.
            for qc in range(QC):
                av = q_slot(qc + 1) if qc < QC - 1 else None
                for j in range(NPAIR):
                    slot(qc, 0, j, av=av)
            # Partner-half blocks: P0 bare; av(qc) rides inside P(qc+1);
            # each chunk's rowsum matmul burst follows its last score block.
            for j in range(NPAIR):
                slot(0, 1, j)
            rowsum(0)
            bias2_compute()
            for qc in range(1, QC):
                avfn = av_slot(qc - 1)
                for j in range(NPAIR):
                    slot(qc, 1, j, av=avfn)
                rowsum(qc)
            # Chunk 3's AV tail runs after the last score block.
            avfn = av_slot(QC - 1)
            for j in range(2 * ET):
                avfn(j)

    _split_excess_waits(nc, mybir)
    return nc


def _get_nc():
    if "nc" not in _CACHE:
        _CACHE["nc"] = _build_nc()
    return _CACHE["nc"]


def _make_in_maps(x, Wq, bq, Wk, bk, Wv, bv):
    bf16 = ml_dtypes.bfloat16
    def _retile(W):
        # [P, DT*D] with w_re[p, d*D+j] = W.T[d*P+p, j]: per-partition
        # contiguous 4KB runs so the whole matrix loads as one DMA.
        return np.ascontiguousarray(
            W.T.reshape(DT, P, D).transpose(1, 0, 2).reshape(P, DT * D)
        ).astype(bf16)

    wqT = _retile(Wq)
    wkT = _retile(Wk)
    wvT = _retile(Wv)
    bqp = np.ascontiguousarray(bq.reshape(ET, P).T).astype(np.float32)
    bkp = np.ascontiguousarray(bk.reshape(ET, P).T).astype(np.float32)
    bvp = np.ascontiguousarray(bv.reshape(ET, P).T).astype(np.float32)
    in_maps = []
    for c in range(NCORES):
        b, h = divmod(c, 2)
        # Local half of x[b].T: both this core's query columns and its K/V
        # half (they are the same row range by construction).
        xTl = np.ascontiguousarray(x[b, h * SQ : (h + 1) * SQ, :].T).astype(bf16)
        # Partner-half row bases into the rank-ordered AllGather outputs.
        poff = np.array(
            [[(1 - h) * ET * P, (1 - h) * HKT * P]], dtype=np.uint32
        )
        in_maps.append(
            {
                "xT": xTl,
                "poff": poff,
                "wqT": wqT,
                "wkT": wkT,
                "wvT": wvT,
                "bq": bqp,
                "bk": bkp,
                "bv": bvp,
            }
        )
    return in_maps


def _run(in_maps, **kwargs):
    from concourse.bass_utils import run_bass_kernel_spmd

    nc = _get_nc()
    return run_bass_kernel_spmd(nc, in_maps, core_ids=list(range(NCORES)), **kwargs)


def kernel(x, Wq, bq, Wk, bk, Wv, bv):
    x = np.asarray(x, dtype=np.float32)
    Wq = np.asarray(Wq, dtype=np.float32)
    Wk = np.asarray(Wk, dtype=np.float32)
    Wv = np.asarray(Wv, dtype=np.float32)
    bq = np.asarray(bq, dtype=np.float32)
    bk = np.asarray(bk, dtype=np.float32)
    bv = np.asarray(bv, dtype=np.float32)

    res = _run(_make_in_maps(x, Wq, bq, Wk, bk, Wv, bv))
    out = np.empty((B, S, D), dtype=np.float32)
    for c in range(NCORES):
        b, h = divmod(c, 2)
        out[b, h * SQ : (h + 1) * SQ, :] = np.asarray(res.results[c]["ot"]).T
    return out


# revision 30
# speedup vs baseline: 1.0367x; 1.0121x over previous
"""BertSelfAttention (B=4, S=4096, D=512) on 8 TRN2 NeuronCores.

Sharding: core c handles batch b = c//2 and query-row half h = c%2
(2048 q rows). K/V are computed on-core for the full 4096 keys of that
batch (halves exchanged within each core pair), avoiding big collectives.

Layout trick: everything is computed transposed so no on-device
transposes are needed:
  QT[e, q] = Wq @ x.T          (lhsT = WqT chunks, rhs = xT chunks)
  KT[e, k] = Wk @ x.T
  V [k, e] = x @ Wv.T          (lhsT = xT chunks,  rhs = WvT)
  ST[k, q] = K Q.T             (lhsT = KT chunks,  rhs = QT)   -> exp -> PT
  OT[e, q] = V.T P.T           (lhsT = V chunks,   rhs = PT)
Softmax runs without max-subtraction (scores are ~N(0, 0.3^2), so exp
cannot overflow and the result is mathematically identical).

Precision: projections run in bf16 (fp32 PSUM); Q/K/V/P are quantized
to fp8e4 (e4m3) and the two big matmuls run as fp8 DoubleRow matmuls
(256-deep contraction per pass = 2x bf16 MAC rate on HW). The rank-1
component of the V-quantization error (softmax rows sum to one) is
corrected by adding (colsum(V_f32) - colsum(V_fp8))/S to the output
bias; colsum(V_f32) accumulates on the DVE during the V projection and
colsum(V_fp8) is an all-ones fp8 matmul, so the correction costs ~2us.

Schedule (trace-driven rewrite of the 209.6us baseline; ~205.2us):
- v1 put the softmax row-sums on the DVE (2 adds/pair): the DVE fell
  ~9us behind the exp stream, and its rowsum -> cast -> ones-matmul ->
  3.4us-reciprocal chain stalled the PE twice (8.9us + 4.3us) with
  clock-droop penalties on each restart. Here the row-sums are 16
  all-ones fp8 DoubleRow matmuls per chunk (+216ns/pair on the PE,
  partition-replicated result in one PSUM bank), emitted right after
  the chunk's partner score block; the reciprocal follows immediately
  and the DVE stays ~60% idle. (GpSimd tensor ops measured 3-15x
  slower than the cost model — eff ~0.05 for tensor_scalar — so
  offloading element work to Pool is a dead end on this HW.)
- AV matmuls for q-chunk qc are interleaved per-pair into the NEXT
  chunk's partner-half score block (8 av matmuls per score slot,
  e-major), so the PE runs back-to-back instead of alternating
  exp-paced score stretches with pure-AV stretches. Evacuation:
  DVE normalize-multiply, ACT bias-add (emitted after the slot's exp
  so the ACT queue stays exps-in-order), DMA out on alternating
  queues; the final chunk evacuates in 256-col halves to shorten the
  drain tail.
- K and V projections are fused per x column-chunk (4 KT e-tiles + 4 V
  k-tiles per chunk, ~7us of PE work per 512KB of arriving x), so the
  PE rides just behind the input DMA stream instead of stalling on it.
  Q chunks 1-3 ride inside the local score blocks (2 matmuls/slot,
  DVE evacuation); only K, V and Q-chunk-0 gate attention start.
- Inputs load via the 3 DMA-capable queues (SP/GpSimd/ACT) as whole-
  tensor transfers with 4KB-contiguous runs per partition (weights are
  host-retiled to [128, 4x512] blocks), sustaining ~330GB/s vs ~250
  for 1KB column-sliced loads; wk+wv first, then the x d-chunks; ~96
  throwaway matmuls warm the PE clock gate during the DMA wait.
  bias2 computes on GpSimd so its wait on the late s AllGather cannot
  back up the DVE mult/reciprocal stream.
- Collective order matters: KT, then V, then the tiny s exchange —
  inserting s before V costs a full collective-startup latency on the
  1MB V gather and starves the AV phase (measured +37us).
- PSUM: 2x2-bank score tiles, a 3-buffer [128,512] pool shared by AV
  accumulators and Q-projection tiles, and 1 rowsum bank (8/8 banks).
"""

import sys

for _p in ("/opt/trn_rl_repo", "/root/.axon_site/_ro/trn_rl_repo"):
    if _p not in sys.path:
        sys.path.append(_p)

import numpy as np
import ml_dtypes

B, S, D = 4, 4096, 512
NCORES = 8
SQ = S // 2  # query rows per core
P = 128
NQ = 512  # q-chunk width (moving free dim)
DT = D // P  # 4 contraction chunks for d
ET = D // P  # 4 e tiles
KTI = S // P  # 32 k tiles
QC = SQ // NQ  # 4 q chunks per core
HKT = KTI // 2  # 16 local k-tiles per core
HS = S // 2  # 2048 local keys per core
NPAIR = HKT // 2  # 8 k-tile pairs per half-block
SCALE = 1.0 / float(np.sqrt(np.float32(D)))
NWARM = 144

_CACHE = {}


def _split_excess_waits(nc, mybir, max_waits=1):
    """This walrus build rejects instructions carrying more than a couple of
    sync waits. Cap every instruction at `max_waits`, spilling the rest onto
    same-engine InstNoOps inserted immediately before it (equivalent
    semantics: the engine's stream stalls at the nop instead)."""
    for f in nc.m.functions:
        for bb in f.blocks:
            old = list(bb.instructions)
            new = []
            for inst in old:
                si = inst.sync_info
                waits = list(si.on_wait) if si is not None and si.on_wait else []
                if len(waits) > max_waits:
                    keep = waits[-max_waits:]
                    excess = waits[:-max_waits]
                    for i in range(0, len(excess), max_waits):
                        nop = mybir.InstNoOp(
                            name=f"waitnop-{nc.next_id()}", ins=[], outs=[]
                        )
                        nop.engine = inst.engine
                        nop.sync_info = mybir.SyncInfo(
                            on_wait=excess[i : i + max_waits], on_update=[]
                        )
                        new.append(nop)
                    inst.sync_info = mybir.SyncInfo(
                        on_wait=keep,
                        on_update=list(si.on_update) if si.on_update else [],
                    )
                new.append(inst)
            if len(new) != len(old):
                bb.instructions[:] = new


def _build_nc():
    import concourse.bass as bass
    import concourse.mybir as mybir
    import concourse.tile as tile
    from contextlib import ExitStack

    bf = mybir.dt.bfloat16
    f32 = mybir.dt.float32
    f8 = mybir.dt.float8e4
    AF = mybir.ActivationFunctionType
    DR = mybir.MatmulPerfMode.DoubleRow
    ALU = mybir.AluOpType

    u32 = mybir.dt.uint32
    nc = bass.Bass()
    xT = nc.declare_dram_parameter("xT", [D, SQ], bf, isOutput=False)
    # Weights host-retiled to [P, DT*D]: w_re[p, d*D+j] = wT[d*P+p, j], so
    # each partition's data is one contiguous 4KB run and the whole matrix
    # loads as a single high-throughput DMA.
    wqT = nc.declare_dram_parameter("wqT", [P, DT * D], bf, isOutput=False)
    wkT = nc.declare_dram_parameter("wkT", [P, DT * D], bf, isOutput=False)
    wvT = nc.declare_dram_parameter("wvT", [P, DT * D], bf, isOutput=False)
    bqp = nc.declare_dram_parameter("bq", [P, ET], f32, isOutput=False)
    bkp = nc.declare_dram_parameter("bk", [P, ET], f32, isOutput=False)
    bvp = nc.declare_dram_parameter("bv", [P, ET], f32, isOutput=False)
    # Host-computed row bases into the AllGather outputs for the PARTNER
    # half (rank-dependent: (1-h)*512 + e*128 for KT, (1-h)*2048 + j*128
    # for V). Drives dynamic (register-offset) DMAs.
    poffp = nc.declare_dram_parameter("poff", [1, 2], u32, isOutput=False)
    ot = nc.declare_dram_parameter("ot", [D, SQ], f32, isOutput=True)

    with tile.TileContext(nc) as tc, ExitStack() as ctx:
        const_pool = ctx.enter_context(tc.tile_pool(name="const", bufs=1))
        persist = ctx.enter_context(tc.tile_pool(name="persist", bufs=1))
        outp = ctx.enter_context(tc.tile_pool(name="outp", bufs=2))
        xin_pool = ctx.enter_context(tc.tile_pool(name="xin", bufs=1))

        ones = const_pool.tile([P, P], bf, tag="ones")
        nc.vector.memset(ones, 1.0)
        ones_f8 = const_pool.tile([P, 1], f8, tag="ones8")
        nc.gpsimd.memset(ones_f8, 1.0)
        # Wide all-ones fp8 stationary for the DoubleRow rowsum matmuls
        # (partition-reduces a PT pair and replicates across partitions).
        ones_f8w = const_pool.tile([P, 2, P], f8, tag="ones8w")
        nc.gpsimd.memset(ones_f8w, 1.0)
        bq_sb = const_pool.tile([P, ET], f32, tag="bq")
        bk_sb = const_pool.tile([P, ET], f32, tag="bk")
        bv_sb = const_pool.tile([P, ET], f32, tag="bv")
        wq_sb = const_pool.tile([P, DT, D], bf, tag="wq", name="wq")
        wk_sb = const_pool.tile([P, DT, D], bf, tag="wk", name="wk")
        wv_sb = const_pool.tile([P, DT, D], bf, tag="wv", name="wv")
        # fp8 operand tiles for the DoubleRow matmuls; contraction-paired
        # chunks live in dim 1 so [:, e:e+2, cols] is a valid 3D AP.
        qt_sb = persist.tile([P, ET, SQ], f8, tag="qt", name="qt")
        # K/V k-order per core: [my half, partner half]. Separate tiles per
        # half so partner DMA-writes create no false deps on local reads.
        kt_loc = persist.tile([P, ET, HS], f8, tag="ktl", name="ktl")
        kt_rem = persist.tile([P, ET, HS], f8, tag="ktr", name="ktr")
        v_loc = persist.tile([P, HKT, D], f8, tag="vl", name="vl")
        v_rem = persist.tile([P, HKT, D], f8, tag="vr", name="vr")
        poff_sb = const_pool.tile([1, 2], u32, tag="poff")
        bias2 = const_pool.tile([P, ET], f32, tag="bias2")

        # ---- Phase 1+2: load inputs on 4 DMA queues, project K + local V,
        # AllGather the K/V halves within each core pair, project Q chunk 0.
        with (
            tc.tile_pool(name="psA", bufs=4, space="PSUM") as psA,
            tc.tile_pool(name="dram", bufs=1, space="DRAM") as dram,
        ):
            ktl_d = dram.tile([ET * P, HS], f8, tag="ktl_d")
            ktg_d = dram.tile([2 * ET * P, HS], f8, tag="ktg_d")
            vl_d = dram.tile([HKT * P, D], f8, tag="vl_d")
            vg_d = dram.tile([2 * HKT * P, D], f8, tag="vg_d")
            sl_d = dram.tile([1, D], f32, tag="sl_d")
            sg_d = dram.tile([2, D], f32, tag="sg_d")

            x_sb = [xin_pool.tile([P, HS], bf, tag=f"x{d}", name=f"x{d}") for d in range(DT)]
            # 3 parallel hardware DMA queues (SP, GpSimd, ACT — the only
            # DMA-capable engines); per queue: wk chunks first (the first
            # projection needs them), then x column-chunks in first-consumer
            # order, then the later-needed weights/biases.
            qeng = [nc.sync, nc.gpsimd, nc.scalar]
            # Whole-tensor transfers with 4KB-contiguous runs per partition:
            # wk + wv upfront (the fused K/V projection needs both), the
            # four x d-chunks (full 2048-col rows), then the late-needed wq.
            qeng[0].dma_start(
                out=wk_sb, in_=wkT[:, :].rearrange("p (d c) -> p d c", d=DT)
            )
            qeng[1].dma_start(
                out=wv_sb, in_=wvT[:, :].rearrange("p (d c) -> p d c", d=DT)
            )
            for d in range(DT):
                qeng[(2 + d) % 3].dma_start(
                    out=x_sb[d], in_=xT[d * P : (d + 1) * P, :]
                )
            qeng[1].dma_start(
                out=wq_sb, in_=wqT[:, :].rearrange("p (d c) -> p d c", d=DT)
            )
            qeng[1].dma_start(out=bk_sb, in_=bkp[:, :])
            qeng[2].dma_start(out=bq_sb, in_=bqp[:, :])
            qeng[2].dma_start(out=bv_sb, in_=bvp[:, :])
            qeng[0].dma_start(out=poff_sb, in_=poffp[:, :])

            # Warm the PE HAM clock gate (~3.4us of activity flips it from
            # 1.2 to 2.4 GHz) with throwaway matmuls while the first input
            # DMAs are still in flight.
            warm_ps = psA.tile([P, P], f32, tag="warm", name="warm_ps", bufs=1)
            for _ in range(NWARM):
                nc.tensor.matmul(warm_ps, lhsT=ones, rhs=ones, start=True, stop=True)

            # Fused K+V projection, one x column-chunk at a time: each
            # arriving 512-col x chunk unlocks ~7us of PE work (4 KT e-tiles
            # + 4 V k-tiles), so the PE never waits on the x DMA stream.
            # K bias fused on ACT evacuation; V copied to fp8 on ACT with
            # the DVE accumulating colsum(V_f32) for the rank-1 correction.
            vacc = outp.tile([P, D], f32, tag="vacc", bufs=1)
            for kc in range(QC):
                for e in range(ET):
                    ps = psA.tile([P, NQ], f32, tag="ps")
                    for d in range(DT):
                        nc.tensor.matmul(
                            ps,
                            lhsT=wk_sb[:, d, e * P : (e + 1) * P],
                            rhs=x_sb[d][:, kc * NQ : (kc + 1) * NQ],
                            start=(d == 0),
                            stop=(d == DT - 1),
                        )
                    nc.scalar.activation(
                        out=kt_loc[:, e, kc * NQ : (kc + 1) * NQ],
                        in_=ps,
                        func=AF.Identity,
                        bias=bk_sb[:, e : e + 1],
                        scale=1.0,
                    )
                for kk in range(4):
                    k = 4 * kc + kk
                    ps = psA.tile([P, D], f32, tag="ps")
                    for d in range(DT):
                        nc.tensor.matmul(
                            ps,
                            lhsT=x_sb[d][:, k * P : (k + 1) * P],
                            rhs=wv_sb[:, d, :],
                            start=(d == 0),
                            stop=(d == DT - 1),
                        )
                    nc.scalar.copy(out=v_loc[:, k, :], in_=ps)
                    if k == 0:
                        nc.vector.tensor_copy(out=vacc, in_=ps)
                    else:
                        nc.vector.tensor_add(vacc, vacc, ps)
                    qeng[k % 3].dma_start(out=vl_d[k * P : (k + 1) * P, :], in_=v_loc[:, k, :])
            for e in range(ET):
                qeng[2 - (e % 2)].dma_start(
                    out=ktl_d[e * P : (e + 1) * P, :], in_=kt_loc[:, e, :]
                )
            # Start the KT exchange immediately: the CC engine is idle and
            # the partner half gates the partner-score blocks.
            pairs = [[2 * i, 2 * i + 1] for i in range(NCORES // 2)]
            nc.gpsimd.collective_compute(
                "AllGather",
                mybir.AluOpType.bypass,
                replica_groups=pairs,
                ins=[ktl_d.opt()],
                outs=[ktg_d.opt()],
            )
            nc.gpsimd.collective_compute(
                "AllGather",
                mybir.AluOpType.bypass,
                replica_groups=pairs,
                ins=[vl_d.opt()],
                outs=[vg_d.opt()],
            )
            # s_local = colsum(V_f32) - colsum(V_fp8): partition-reduce vacc
            # with a ones-matmul; colsum the fp8 tiles with an all-ones fp8
            # DoubleRow matmul (exact f32 accumulation).
            vacc_bf = outp.tile([P, D], bf, tag="vacc_bf", bufs=1)
            nc.vector.tensor_copy(out=vacc_bf, in_=vacc)
            psc = psA.tile([1, D], f32, tag="c1", bufs=1)
            nc.tensor.matmul(psc, lhsT=ones[:, 0:1], rhs=vacc_bf, start=True, stop=True)
            # colsum(V_fp8) via the wide all-ones DoubleRow matmul: the
            # result is partition-replicated, row 0 is the exact f32 colsum.
            psv = psA.tile([P, D], f32, tag="c2", bufs=1)
            for k in range(0, HKT, 2):
                nc.tensor.matmul(
                    psv,
                    lhsT=ones_f8w,
                    rhs=v_loc[:, k : k + 2, :],
                    start=(k == 0),
                    stop=(k == HKT - 2),
                    perf_mode=DR,
                )
            sl_c = outp.tile([1, D], f32, tag="sl_c", bufs=1)
            nc.scalar.copy(out=sl_c, in_=psc)
            sl_sb = outp.tile([1, D], f32, tag="sl_sb", bufs=1)
            nc.vector.tensor_sub(sl_sb, sl_c, psv[0:1, :])
            nc.sync.dma_start(out=sl_d, in_=sl_sb)
            nc.gpsimd.collective_compute(
                "AllGather",
                mybir.AluOpType.bypass,
                replica_groups=pairs,
                ins=[sl_d.opt()],
                outs=[sg_d.opt()],
            )

            # Partner-half loads from the gather outputs, issued before the
            # Q projection so the transfers start the moment each gather
            # lands. The row base is rank-dependent, supplied by the host
            # via `poff` and applied as a dynamic (register) offset.
            SP = [mybir.EngineType.SP]
            kt_base = nc.values_load(
                poff_sb[0:1, 0:1], engines=SP,
                min_val=0, max_val=ET * P,
                skip_runtime_bounds_check=True,
            )
            nc.sync.dma_start(
                out=kt_rem,
                in_=ktg_d[bass.ds(kt_base, ET * P), :].rearrange(
                    "(e p) c -> p e c", p=P
                ),
            )
            v_base = nc.values_load(
                poff_sb[0:1, 1:2], engines=SP,
                min_val=0, max_val=HKT * P,
                skip_runtime_bounds_check=True,
            )
            nc.sync.dma_start(
                out=v_rem,
                in_=vg_d[bass.ds(v_base, HKT * P), :].rearrange(
                    "(j p) c -> p j c", p=P
                ),
            )
            s_a = outp.tile([P, ET], f32, tag="s_a", bufs=1)
            s_b = outp.tile([P, ET], f32, tag="s_b", bufs=1)
            nc.sync.dma_start(
                out=s_a, in_=sg_d[0:1, :].rearrange("r (et p) -> (r p) et", p=P)
            )
            nc.sync.dma_start(
                out=s_b, in_=sg_d[1:2, :].rearrange("r (et p) -> (r p) et", p=P)
            )

        # ---- Phase 3: attention ----
        with (
            tc.tile_pool(name="pt", bufs=1) as pt_pool,
            tc.tile_pool(name="ps_st", bufs=2, space="PSUM") as ps_st,
            tc.tile_pool(name="ps_ot", bufs=3, space="PSUM") as ps_ot,
            tc.tile_pool(name="ps_rs", bufs=1, space="PSUM") as ps_rs,
        ):
            ptl_tiles = {}
            ptp_tiles = {}
            recips = {}

            # Q projection: chunk 0 eagerly (ACT evacuation: the exp stream
            # hasn't started); chunks 1-3 interleaved into the local score
            # blocks with DVE evacuation.
            def q_mms(qc, e, d0, d1, ps):
                for d in range(d0, d1):
                    nc.tensor.matmul(
                        ps,
                        lhsT=wq_sb[:, d, e * P : (e + 1) * P],
                        rhs=x_sb[d][:, qc * NQ : (qc + 1) * NQ],
                        start=(d == 0),
                        stop=(d == DT - 1),
                    )

            for e in range(ET):
                ps = ps_ot.tile([P, NQ], f32, tag="ot", name=f"q0e{e}")
                q_mms(0, e, 0, DT, ps)
                nc.scalar.activation(
                    out=qt_sb[:, e, 0:NQ],
                    in_=ps,
                    func=AF.Identity,
                    bias=bq_sb[:, e : e + 1],
                    scale=1.0,
                )

            def pt_half(qc, half):
                return ptl_tiles[qc] if half == 0 else ptp_tiles[qc]

            q_ps = {}

            def slot(qc, half, j, av=None):
                """One k-tile-pair score slot: 4 DR matmuls -> exp -> rowsum
                (alternating DVE/GpSimd). `av` optionally appends AV or
                Q-projection matmuls to the PE stream inside this slot."""
                qsl = slice(qc * NQ, (qc + 1) * NQ)
                if j == 0:
                    t = pt_pool.tile(
                        [P, HKT, NQ], f8,
                        tag=("ptl" if half == 0 else "ptp"),
                        name=f"pt{half}_{qc}",
                        bufs=(4 if half == 0 else 3),
                    )
                    if half == 0:
                        ptl_tiles[qc] = t
                    else:
                        ptp_tiles[qc] = t
                ps = ps_st.tile([P, 2, NQ], f32, tag="st", name="st_ps")
                for kh in range(2):
                    kk = 2 * j + kh
                    for e in (0, 2):
                        if half == 0:
                            lhsT = kt_loc[:, e : e + 2, kk * P : (kk + 1) * P]
                        else:
                            lhsT = kt_rem[:, e : e + 2, kk * P : (kk + 1) * P]
                        nc.tensor.matmul(
                            ps[:, kh, :],
                            lhsT=lhsT,
                            rhs=qt_sb[:, e : e + 2, qsl],
                            start=(e == 0),
                            stop=(e == 2),
                            perf_mode=DR,
                        )
                pt_pair = pt_half(qc, half)[:, 2 * j : 2 * j + 2, :]
                nc.scalar.activation(out=pt_pair, in_=ps, func=AF.Exp, scale=SCALE)
                # Interleaved work (AV or Q-projection matmuls) is emitted
                # after the exp so the ACT queue stays a pure exp stream.
                if av is not None:
                    av(j)

            def rowsum(qc):
                # Rowsums on the PE: 16 all-ones fp8 DoubleRow matmuls over
                # the chunk's PT pairs, accumulated in one PSUM bank (the
                # result is partition-replicated), then the reciprocal.
                # Emitted right after P(qc): the last pair's exp completes
                # ~2us into the 3.5us matmul burst, so the PE barely waits.
                rs_ps = ps_rs.tile([P, NQ], f32, tag="rs", name=f"rs{qc}")
                for p in range(2 * NPAIR):
                    if p < NPAIR:
                        prhs = ptl_tiles[qc][:, 2 * p : 2 * p + 2, :]
                    else:
                        prhs = ptp_tiles[qc][:, 2 * (p - NPAIR) : 2 * (p - NPAIR) + 2, :]
                    nc.tensor.matmul(
                        rs_ps,
                        lhsT=ones_f8w,
                        rhs=prhs,
                        start=(p == 0),
                        stop=(p == 2 * NPAIR - 1),
                        perf_mode=DR,
                    )
                recips[qc] = outp.tile(
                    [P, NQ], f32, tag="recip", bufs=2, name=f"recip{qc}"
                )
                nc.vector.reciprocal(recips[qc], rs_ps)

            def bias2_compute():
                # bias2 = bv + (s_local + s_partner) / S on the (idle)
                # GpSimd engine: these [128, 4] ops are tiny even at Pool
                # throughput, and their wait on the s AllGather (~110us)
                # cannot back up the DVE mult/reciprocal stream.
                nc.gpsimd.tensor_add(s_a, s_a, s_b)
                for et in range(ET):
                    nc.gpsimd.tensor_scalar(
                        out=bias2[:, et : et + 1],
                        in0=s_a[:, et : et + 1],
                        scalar1=1.0 / S,
                        scalar2=bv_sb[:, et : et + 1],
                        op0=ALU.mult,
                        op1=ALU.add,
                    )

            av_state = {}

            def av_evac(qc, e, ops, halves=1):
                # Normalize on the DVE, bias-add on ACT (emitted after the
                # slot's exp, so it never sits in front of pending exps; the
                # ACT stream has ~10us of slack per AV-carrying block).
                # `halves=2` pipelines the mult/bias/DMA chain in 256-col
                # pieces — used on the final chunk to shorten the tail.
                tmp = outp.tile([P, NQ], f32, tag="tmp", bufs=3)
                hw = NQ // halves
                for hh in range(halves):
                    hs = slice(hh * hw, (hh + 1) * hw)
                    qsl = slice(qc * NQ + hh * hw, qc * NQ + (hh + 1) * hw)
                    nc.vector.tensor_mul(tmp[:, hs], ops[:, hs], recips[qc][:, hs])
                    nc.scalar.activation(
                        out=tmp[:, hs],
                        in_=tmp[:, hs],
                        func=AF.Identity,
                        bias=bias2[:, e : e + 1],
                        scale=1.0,
                    )
                    (nc.sync if (e + hh) % 2 == 0 else nc.gpsimd).dma_start(
                        out=ot[e * P : (e + 1) * P, qsl], in_=tmp[:, hs]
                    )

            def av_mms(qc, e, ops, p0, p1):
                for p in range(p0, p1):
                    if p < NPAIR:
                        vlhsT = v_loc[:, 2 * p : 2 * p + 2, e * P : (e + 1) * P]
                        prhs = ptl_tiles[qc][:, 2 * p : 2 * p + 2, :]
                    else:
                        pp = p - NPAIR
                        vlhsT = v_rem[:, 2 * pp : 2 * pp + 2, e * P : (e + 1) * P]
                        prhs = ptp_tiles[qc][:, 2 * pp : 2 * pp + 2, :]
                    nc.tensor.matmul(
                        ops,
                        lhsT=vlhsT,
                        rhs=prhs,
                        start=(p == 0),
                        stop=(p == 2 * NPAIR - 1),
                        perf_mode=DR,
                    )

            def av_slot(qc):
                # 8 AV matmuls per score slot, e-major: e-group e occupies
                # slots 2e (pairs 0-7) and 2e+1 (pairs 8-15 + evacuation).
                def fn(j):
                    e = j // 2
                    if j % 2 == 0:
                        av_state[qc] = ps_ot.tile(
                            [P, NQ], f32, tag="ot", name=f"av{qc}e{e}"
                        )
                        av_mms(qc, e, av_state[qc], 0, NPAIR)
                    else:
                        av_mms(qc, e, av_state[qc], NPAIR, 2 * NPAIR)
                        av_evac(qc, e, av_state[qc], halves=(2 if qc == QC - 1 else 1))
                return fn

            def q_slot(qc):
                # Q-projection for chunk qc rides in the previous local
                # score block, 2 matmuls per slot (e = j//2), with DVE
                # evacuation so the ACT exp stream is untouched.
                def fn(j):
                    e = j // 2
                    if j % 2 == 0:
                        q_ps[qc] = ps_ot.tile([P, NQ], f32, tag="ot", name=f"q{qc}e{e}")
                        q_mms(qc, e, 0, 2, q_ps[qc])
                    else:
                        q_mms(qc, e, 2, DT, q_ps[qc])
                        nc.vector.tensor_scalar(
                            out=qt_sb[:, e, qc * NQ : (qc + 1) * NQ],
                            in0=q_ps[qc],
                            scalar1=bq_sb[:, e : e + 1],
                            scalar2=None,
                            op0=ALU.add,
                        )
                return fn

            # Local-half score blocks, with Q chunks 1-3 interleaved.
            for qc in range(QC):
                av = q_slot(qc + 1) if qc < QC - 1 else None
                for j in range(NPAIR):
                    slot(qc, 0, j, av=av)
            # Partner-half blocks: P0 bare; av(qc) rides inside P(qc+1);
            # each chunk's rowsum matmul burst follows its last score block.
            for j in range(NPAIR):
                slot(0, 1, j)
            rowsum(0)
            bias2_compute()
            for qc in range(1, QC):
                avfn = av_slot(qc - 1)
                for j in range(NPAIR):
                    slot(qc, 1, j, av=avfn)
                rowsum(qc)
            # Chunk 3's AV tail runs after the last score block.
            avfn = av_slot(QC - 1)
            for j in range(2 * ET):
                avfn(j)

    _split_excess_waits(nc, mybir)
    return nc


def _get_nc():
    if "nc" not in _CACHE:
        _CACHE["nc"] = _build_nc()
    return _CACHE["nc"]


def _make_in_maps(x, Wq, bq, Wk, bk, Wv, bv):
    bf16 = ml_dtypes.bfloat16
    def _retile(W):
        # [P, DT*D] with w_re[p, d*D+j] = W.T[d*P+p, j]: per-partition
        # contiguous 4KB runs so the whole matrix loads as one DMA.
        return np.ascontiguousarray(
            W.T.reshape(DT, P, D).transpose(1, 0, 2).reshape(P, DT * D)
        ).astype(bf16)

    wqT = _retile(Wq)
    wkT = _retile(Wk)
    wvT = _retile(Wv)
    bqp = np.ascontiguousarray(bq.reshape(ET, P).T).astype(np.float32)
    bkp = np.ascontiguousarray(bk.reshape(ET, P).T).astype(np.float32)
    bvp = np.ascontiguousarray(bv.reshape(ET, P).T).astype(np.float32)
    in_maps = []
    for c in range(NCORES):
        b, h = divmod(c, 2)
        # Local half of x[b].T: both this core's query columns and its K/V
        # half (they are the same row range by construction).
        xTl = np.ascontiguousarray(x[b, h * SQ : (h + 1) * SQ, :].T).astype(bf16)
        # Partner-half row bases into the rank-ordered AllGather outputs.
        poff = np.array(
            [[(1 - h) * ET * P, (1 - h) * HKT * P]], dtype=np.uint32
        )
        in_maps.append(
            {
                "xT": xTl,
                "poff": poff,
                "wqT": wqT,
                "wkT": wkT,
                "wvT": wvT,
                "bq": bqp,
                "bk": bkp,
                "bv": bvp,
            }
        )
    return in_maps


def _run(in_maps, **kwargs):
    from concourse.bass_utils import run_bass_kernel_spmd

    nc = _get_nc()
    return run_bass_kernel_spmd(nc, in_maps, core_ids=list(range(NCORES)), **kwargs)


def kernel(x, Wq, bq, Wk, bk, Wv, bv):
    x = np.asarray(x, dtype=np.float32)
    Wq = np.asarray(Wq, dtype=np.float32)
    Wk = np.asarray(Wk, dtype=np.float32)
    Wv = np.asarray(Wv, dtype=np.float32)
    bq = np.asarray(bq, dtype=np.float32)
    bk = np.asarray(bk, dtype=np.float32)
    bv = np.asarray(bv, dtype=np.float32)

    res = _run(_make_in_maps(x, Wq, bq, Wk, bk, Wv, bv))
    out = np.empty((B, S, D), dtype=np.float32)
    for c in range(NCORES):
        b, h = divmod(c, 2)
        out[b, h * SQ : (h + 1) * SQ, :] = np.asarray(res.results[c]["ot"]).T
    return out


# revision 31
# speedup vs baseline: 1.0388x; 1.0021x over previous
"""BertSelfAttention (B=4, S=4096, D=512) on 8 TRN2 NeuronCores.

Sharding: core c handles batch b = c//2 and query-row half h = c%2
(2048 q rows). K/V are computed on-core for the full 4096 keys of that
batch (halves exchanged within each core pair), avoiding big collectives.

Layout trick: everything is computed transposed so no on-device
transposes are needed:
  QT[e, q] = Wq @ x.T          (lhsT = WqT chunks, rhs = xT chunks)
  KT[e, k] = Wk @ x.T
  V [k, e] = x @ Wv.T          (lhsT = xT chunks,  rhs = WvT)
  ST[k, q] = K Q.T             (lhsT = KT chunks,  rhs = QT)   -> exp -> PT
  OT[e, q] = V.T P.T           (lhsT = V chunks,   rhs = PT)
Softmax runs without max-subtraction (scores are ~N(0, 0.3^2), so exp
cannot overflow and the result is mathematically identical).

Precision: projections run in bf16 (fp32 PSUM); Q/K/V/P are quantized
to fp8e4 (e4m3) and the two big matmuls run as fp8 DoubleRow matmuls
(256-deep contraction per pass = 2x bf16 MAC rate on HW). The rank-1
component of the V-quantization error (softmax rows sum to one) is
corrected by adding (colsum(V_f32) - colsum(V_fp8))/S to the output
bias; colsum(V_f32) accumulates on the DVE during the V projection and
colsum(V_fp8) is an all-ones fp8 matmul, so the correction costs ~2us.

Schedule (trace-driven rewrite of the 209.6us baseline; ~205.2us):
- v1 put the softmax row-sums on the DVE (2 adds/pair): the DVE fell
  ~9us behind the exp stream, and its rowsum -> cast -> ones-matmul ->
  3.4us-reciprocal chain stalled the PE twice (8.9us + 4.3us) with
  clock-droop penalties on each restart. Here the row-sums are 16
  all-ones fp8 DoubleRow matmuls per chunk (+216ns/pair on the PE,
  partition-replicated result in one PSUM bank), emitted right after
  the chunk's partner score block; the reciprocal follows immediately
  and the DVE stays ~60% idle. (GpSimd tensor ops measured 3-15x
  slower than the cost model — eff ~0.05 for tensor_scalar — so
  offloading element work to Pool is a dead end on this HW.)
- AV matmuls for q-chunk qc are interleaved per-pair into the NEXT
  chunk's partner-half score block (8 av matmuls per score slot,
  e-major), so the PE runs back-to-back instead of alternating
  exp-paced score stretches with pure-AV stretches. Evacuation:
  DVE normalize-multiply, ACT bias-add (emitted after the slot's exp
  so the ACT queue stays exps-in-order), DMA out on alternating
  queues; the final chunk evacuates in 256-col halves to shorten the
  drain tail.
- K and V projections are fused per x column-chunk (4 KT e-tiles + 4 V
  k-tiles per chunk, ~7us of PE work per 512KB of arriving x), so the
  PE rides just behind the input DMA stream instead of stalling on it.
  Q chunks 1-3 ride inside the local score blocks (2 matmuls/slot,
  DVE evacuation); only K, V and Q-chunk-0 gate attention start.
- Inputs load via the 3 DMA-capable queues (SP/GpSimd/ACT) as whole-
  tensor transfers with 4KB-contiguous runs per partition (weights are
  host-retiled to [128, 4x512] blocks), sustaining ~330GB/s vs ~250
  for 1KB column-sliced loads; wk+wv first, then the x d-chunks; ~96
  throwaway matmuls warm the PE clock gate during the DMA wait.
  bias2 computes on GpSimd so its wait on the late s AllGather cannot
  back up the DVE mult/reciprocal stream.
- Collective order matters: KT, then V, then the tiny s exchange —
  inserting s before V costs a full collective-startup latency on the
  1MB V gather and starves the AV phase (measured +37us).
- PSUM: 2x2-bank score tiles, a 3-buffer [128,512] pool shared by AV
  accumulators and Q-projection tiles, and 1 rowsum bank (8/8 banks).
"""

import sys

for _p in ("/opt/trn_rl_repo", "/root/.axon_site/_ro/trn_rl_repo"):
    if _p not in sys.path:
        sys.path.append(_p)

import numpy as np
import ml_dtypes

B, S, D = 4, 4096, 512
NCORES = 8
SQ = S // 2  # query rows per core
P = 128
NQ = 512  # q-chunk width (moving free dim)
DT = D // P  # 4 contraction chunks for d
ET = D // P  # 4 e tiles
KTI = S // P  # 32 k tiles
QC = SQ // NQ  # 4 q chunks per core
HKT = KTI // 2  # 16 local k-tiles per core
HS = S // 2  # 2048 local keys per core
NPAIR = HKT // 2  # 8 k-tile pairs per half-block
SCALE = 1.0 / float(np.sqrt(np.float32(D)))
NWARM = 144

_CACHE = {}


def _split_excess_waits(nc, mybir, max_waits=1):
    """This walrus build rejects instructions carrying more than a couple of
    sync waits. Cap every instruction at `max_waits`, spilling the rest onto
    same-engine InstNoOps inserted immediately before it (equivalent
    semantics: the engine's stream stalls at the nop instead)."""
    for f in nc.m.functions:
        for bb in f.blocks:
            old = list(bb.instructions)
            new = []
            for inst in old:
                si = inst.sync_info
                waits = list(si.on_wait) if si is not None and si.on_wait else []
                if len(waits) > max_waits:
                    keep = waits[-max_waits:]
                    excess = waits[:-max_waits]
                    for i in range(0, len(excess), max_waits):
                        nop = mybir.InstNoOp(
                            name=f"waitnop-{nc.next_id()}", ins=[], outs=[]
                        )
                        nop.engine = inst.engine
                        nop.sync_info = mybir.SyncInfo(
                            on_wait=excess[i : i + max_waits], on_update=[]
                        )
                        new.append(nop)
                    inst.sync_info = mybir.SyncInfo(
                        on_wait=keep,
                        on_update=list(si.on_update) if si.on_update else [],
                    )
                new.append(inst)
            if len(new) != len(old):
                bb.instructions[:] = new


def _build_nc():
    import concourse.bass as bass
    import concourse.mybir as mybir
    import concourse.tile as tile
    from contextlib import ExitStack

    bf = mybir.dt.bfloat16
    f32 = mybir.dt.float32
    f8 = mybir.dt.float8e4
    AF = mybir.ActivationFunctionType
    DR = mybir.MatmulPerfMode.DoubleRow
    ALU = mybir.AluOpType

    u32 = mybir.dt.uint32
    nc = bass.Bass()
    xT = nc.declare_dram_parameter("xT", [D, SQ], bf, isOutput=False)
    # Weights host-retiled to [P, DT*D]: w_re[p, d*D+j] = wT[d*P+p, j], so
    # each partition's data is one contiguous 4KB run and the whole matrix
    # loads as a single high-throughput DMA.
    wqT = nc.declare_dram_parameter("wqT", [P, DT * D], bf, isOutput=False)
    wkT = nc.declare_dram_parameter("wkT", [P, DT * D], bf, isOutput=False)
    wvT = nc.declare_dram_parameter("wvT", [P, DT * D], bf, isOutput=False)
    bqp = nc.declare_dram_parameter("bq", [P, ET], f32, isOutput=False)
    bkp = nc.declare_dram_parameter("bk", [P, ET], f32, isOutput=False)
    bvp = nc.declare_dram_parameter("bv", [P, ET], f32, isOutput=False)
    # Host-computed row bases into the AllGather outputs for the PARTNER
    # half (rank-dependent: (1-h)*512 + e*128 for KT, (1-h)*2048 + j*128
    # for V). Drives dynamic (register-offset) DMAs.
    poffp = nc.declare_dram_parameter("poff", [1, 2], u32, isOutput=False)
    ot = nc.declare_dram_parameter("ot", [D, SQ], f32, isOutput=True)

    with tile.TileContext(nc) as tc, ExitStack() as ctx:
        const_pool = ctx.enter_context(tc.tile_pool(name="const", bufs=1))
        persist = ctx.enter_context(tc.tile_pool(name="persist", bufs=1))
        outp = ctx.enter_context(tc.tile_pool(name="outp", bufs=2))
        xin_pool = ctx.enter_context(tc.tile_pool(name="xin", bufs=1))

        ones = const_pool.tile([P, P], bf, tag="ones")
        nc.vector.memset(ones, 1.0)
        ones_f8 = const_pool.tile([P, 1], f8, tag="ones8")
        nc.gpsimd.memset(ones_f8, 1.0)
        # Wide all-ones fp8 stationary for the DoubleRow rowsum matmuls
        # (partition-reduces a PT pair and replicates across partitions).
        ones_f8w = const_pool.tile([P, 2, P], f8, tag="ones8w")
        nc.gpsimd.memset(ones_f8w, 1.0)
        bq_sb = const_pool.tile([P, ET], f32, tag="bq")
        bk_sb = const_pool.tile([P, ET], f32, tag="bk")
        bv_sb = const_pool.tile([P, ET], f32, tag="bv")
        wq_sb = const_pool.tile([P, DT, D], bf, tag="wq", name="wq")
        wk_sb = const_pool.tile([P, DT, D], bf, tag="wk", name="wk")
        wv_sb = const_pool.tile([P, DT, D], bf, tag="wv", name="wv")
        # fp8 operand tiles for the DoubleRow matmuls; contraction-paired
        # chunks live in dim 1 so [:, e:e+2, cols] is a valid 3D AP.
        qt_sb = persist.tile([P, ET, SQ], f8, tag="qt", name="qt")
        # K/V k-order per core: [my half, partner half]. Separate tiles per
        # half so partner DMA-writes create no false deps on local reads.
        kt_loc = persist.tile([P, ET, HS], f8, tag="ktl", name="ktl")
        kt_rem = persist.tile([P, ET, HS], f8, tag="ktr", name="ktr")
        v_loc = persist.tile([P, HKT, D], f8, tag="vl", name="vl")
        v_rem = persist.tile([P, HKT, D], f8, tag="vr", name="vr")
        poff_sb = const_pool.tile([1, 2], u32, tag="poff")
        bias2 = const_pool.tile([P, ET], f32, tag="bias2")

        # ---- Phase 1+2: load inputs on 4 DMA queues, project K + local V,
        # AllGather the K/V halves within each core pair, project Q chunk 0.
        with (
            tc.tile_pool(name="psA", bufs=4, space="PSUM") as psA,
            tc.tile_pool(name="dram", bufs=1, space="DRAM") as dram,
        ):
            ktl_d = dram.tile([ET * P, HS], f8, tag="ktl_d")
            ktg_d = dram.tile([2 * ET * P, HS], f8, tag="ktg_d")
            vl_d = dram.tile([HKT * P, D], f8, tag="vl_d")
            vg_d = dram.tile([2 * HKT * P, D], f8, tag="vg_d")
            sl_d = dram.tile([1, D], f32, tag="sl_d")
            sg_d = dram.tile([2, D], f32, tag="sg_d")

            x_sb = [xin_pool.tile([P, HS], bf, tag=f"x{d}", name=f"x{d}") for d in range(DT)]
            # 3 parallel hardware DMA queues (SP, GpSimd, ACT — the only
            # DMA-capable engines); per queue: wk chunks first (the first
            # projection needs them), then x column-chunks in first-consumer
            # order, then the later-needed weights/biases.
            qeng = [nc.sync, nc.gpsimd, nc.scalar]
            # Whole-tensor transfers with 4KB-contiguous runs per partition:
            # wk + wv upfront (the fused K/V projection needs both), the
            # four x d-chunks (full 2048-col rows), then the late-needed wq.
            qeng[0].dma_start(
                out=wk_sb, in_=wkT[:, :].rearrange("p (d c) -> p d c", d=DT)
            )
            qeng[1].dma_start(
                out=wv_sb, in_=wvT[:, :].rearrange("p (d c) -> p d c", d=DT)
            )
            for d in range(DT):
                qeng[(2 + d) % 3].dma_start(
                    out=x_sb[d], in_=xT[d * P : (d + 1) * P, :]
                )
            qeng[1].dma_start(
                out=wq_sb, in_=wqT[:, :].rearrange("p (d c) -> p d c", d=DT)
            )
            qeng[1].dma_start(out=bk_sb, in_=bkp[:, :])
            qeng[2].dma_start(out=bq_sb, in_=bqp[:, :])
            qeng[2].dma_start(out=bv_sb, in_=bvp[:, :])
            qeng[0].dma_start(out=poff_sb, in_=poffp[:, :])

            # Warm the PE HAM clock gate (~3.4us of activity flips it from
            # 1.2 to 2.4 GHz) with throwaway matmuls while the first input
            # DMAs are still in flight.
            warm_ps = psA.tile([P, P], f32, tag="warm", name="warm_ps", bufs=1)
            for _ in range(NWARM):
                nc.tensor.matmul(warm_ps, lhsT=ones, rhs=ones, start=True, stop=True)

            # Fused K+V projection, one x column-chunk at a time: each
            # arriving 512-col x chunk unlocks ~7us of PE work (4 KT e-tiles
            # + 4 V k-tiles), so the PE never waits on the x DMA stream.
            # K bias fused on ACT evacuation; V copied to fp8 on ACT with
            # the DVE accumulating colsum(V_f32) for the rank-1 correction.
            vacc = outp.tile([P, D], f32, tag="vacc", bufs=1)
            for kc in range(QC):
                for e in range(ET):
                    ps = psA.tile([P, NQ], f32, tag="ps")
                    for d in range(DT):
                        nc.tensor.matmul(
                            ps,
                            lhsT=wk_sb[:, d, e * P : (e + 1) * P],
                            rhs=x_sb[d][:, kc * NQ : (kc + 1) * NQ],
                            start=(d == 0),
                            stop=(d == DT - 1),
                        )
                    nc.scalar.activation(
                        out=kt_loc[:, e, kc * NQ : (kc + 1) * NQ],
                        in_=ps,
                        func=AF.Identity,
                        bias=bk_sb[:, e : e + 1],
                        scale=1.0,
                    )
                for kk in range(4):
                    k = 4 * kc + kk
                    ps = psA.tile([P, D], f32, tag="ps")
                    for d in range(DT):
                        nc.tensor.matmul(
                            ps,
                            lhsT=x_sb[d][:, k * P : (k + 1) * P],
                            rhs=wv_sb[:, d, :],
                            start=(d == 0),
                            stop=(d == DT - 1),
                        )
                    nc.scalar.copy(out=v_loc[:, k, :], in_=ps)
                    if k == 0:
                        nc.vector.tensor_copy(out=vacc, in_=ps)
                    else:
                        nc.vector.tensor_add(vacc, vacc, ps)
                    qeng[k % 3].dma_start(out=vl_d[k * P : (k + 1) * P, :], in_=v_loc[:, k, :])
            for e in range(ET):
                qeng[2 - (e % 2)].dma_start(
                    out=ktl_d[e * P : (e + 1) * P, :], in_=kt_loc[:, e, :]
                )
            # Start the KT exchange immediately: the CC engine is idle and
            # the partner half gates the partner-score blocks.
            pairs = [[2 * i, 2 * i + 1] for i in range(NCORES // 2)]
            nc.gpsimd.collective_compute(
                "AllGather",
                mybir.AluOpType.bypass,
                replica_groups=pairs,
                ins=[ktl_d.opt()],
                outs=[ktg_d.opt()],
            )
            nc.gpsimd.collective_compute(
                "AllGather",
                mybir.AluOpType.bypass,
                replica_groups=pairs,
                ins=[vl_d.opt()],
                outs=[vg_d.opt()],
            )
            # Q chunk 0 projected here, before the colsum matmuls: its ACT
            # evacuations then overlap the colsum work instead of gating the
            # first score block. (Chunks 1-3 ride inside the score blocks.)
            for e in range(ET):
                ps = psA.tile([P, NQ], f32, tag="ps")
                for d in range(DT):
                    nc.tensor.matmul(
                        ps,
                        lhsT=wq_sb[:, d, e * P : (e + 1) * P],
                        rhs=x_sb[d][:, 0:NQ],
                        start=(d == 0),
                        stop=(d == DT - 1),
                    )
                nc.scalar.activation(
                    out=qt_sb[:, e, 0:NQ],
                    in_=ps,
                    func=AF.Identity,
                    bias=bq_sb[:, e : e + 1],
                    scale=1.0,
                )
            # s_local = colsum(V_f32) - colsum(V_fp8): partition-reduce vacc
            # with a ones-matmul; colsum the fp8 tiles with an all-ones fp8
            # DoubleRow matmul (exact f32 accumulation).
            vacc_bf = outp.tile([P, D], bf, tag="vacc_bf", bufs=1)
            nc.vector.tensor_copy(out=vacc_bf, in_=vacc)
            psc = psA.tile([1, D], f32, tag="c1", bufs=1)
            nc.tensor.matmul(psc, lhsT=ones[:, 0:1], rhs=vacc_bf, start=True, stop=True)
            # colsum(V_fp8) via the wide all-ones DoubleRow matmul: the
            # result is partition-replicated, row 0 is the exact f32 colsum.
            psv = psA.tile([P, D], f32, tag="c2", bufs=1)
            for k in range(0, HKT, 2):
                nc.tensor.matmul(
                    psv,
                    lhsT=ones_f8w,
                    rhs=v_loc[:, k : k + 2, :],
                    start=(k == 0),
                    stop=(k == HKT - 2),
                    perf_mode=DR,
                )
            sl_c = outp.tile([1, D], f32, tag="sl_c", bufs=1)
            nc.scalar.copy(out=sl_c, in_=psc)
            sl_sb = outp.tile([1, D], f32, tag="sl_sb", bufs=1)
            nc.vector.tensor_sub(sl_sb, sl_c, psv[0:1, :])
            nc.sync.dma_start(out=sl_d, in_=sl_sb)
            nc.gpsimd.collective_compute(
                "AllGather",
                mybir.AluOpType.bypass,
                replica_groups=pairs,
                ins=[sl_d.opt()],
                outs=[sg_d.opt()],
            )

            # Partner-half loads from the gather outputs, issued before the
            # Q projection so the transfers start the moment each gather
            # lands. The row base is rank-dependent, supplied by the host
            # via `poff` and applied as a dynamic (register) offset.
            SP = [mybir.EngineType.SP]
            kt_base = nc.values_load(
                poff_sb[0:1, 0:1], engines=SP,
                min_val=0, max_val=ET * P,
                skip_runtime_bounds_check=True,
            )
            nc.sync.dma_start(
                out=kt_rem,
                in_=ktg_d[bass.ds(kt_base, ET * P), :].rearrange(
                    "(e p) c -> p e c", p=P
                ),
            )
            v_base = nc.values_load(
                poff_sb[0:1, 1:2], engines=SP,
                min_val=0, max_val=HKT * P,
                skip_runtime_bounds_check=True,
            )
            nc.sync.dma_start(
                out=v_rem,
                in_=vg_d[bass.ds(v_base, HKT * P), :].rearrange(
                    "(j p) c -> p j c", p=P
                ),
            )
            s_a = outp.tile([P, ET], f32, tag="s_a", bufs=1)
            s_b = outp.tile([P, ET], f32, tag="s_b", bufs=1)
            nc.sync.dma_start(
                out=s_a, in_=sg_d[0:1, :].rearrange("r (et p) -> (r p) et", p=P)
            )
            nc.sync.dma_start(
                out=s_b, in_=sg_d[1:2, :].rearrange("r (et p) -> (r p) et", p=P)
            )

        # ---- Phase 3: attention ----
        with (
            tc.tile_pool(name="pt", bufs=1) as pt_pool,
            tc.tile_pool(name="ps_st", bufs=2, space="PSUM") as ps_st,
            tc.tile_pool(name="ps_ot", bufs=3, space="PSUM") as ps_ot,
            tc.tile_pool(name="ps_rs", bufs=1, space="PSUM") as ps_rs,
        ):
            ptl_tiles = {}
            ptp_tiles = {}
            recips = {}

            # Q projection: chunk 0 eagerly (ACT evacuation: the exp stream
            # hasn't started); chunks 1-3 interleaved into the local score
            # blocks with DVE evacuation.
            def q_mms(qc, e, d0, d1, ps):
                for d in range(d0, d1):
                    nc.tensor.matmul(
                        ps,
                        lhsT=wq_sb[:, d, e * P : (e + 1) * P],
                        rhs=x_sb[d][:, qc * NQ : (qc + 1) * NQ],
                        start=(d == 0),
                        stop=(d == DT - 1),
                    )

            def pt_half(qc, half):
                return ptl_tiles[qc] if half == 0 else ptp_tiles[qc]

            q_ps = {}

            def slot(qc, half, j, av=None):
                """One k-tile-pair score slot: 4 DR matmuls -> exp -> rowsum
                (alternating DVE/GpSimd). `av` optionally appends AV or
                Q-projection matmuls to the PE stream inside this slot."""
                qsl = slice(qc * NQ, (qc + 1) * NQ)
                if j == 0:
                    t = pt_pool.tile(
                        [P, HKT, NQ], f8,
                        tag=("ptl" if half == 0 else "ptp"),
                        name=f"pt{half}_{qc}",
                        bufs=(4 if half == 0 else 3),
                    )
                    if half == 0:
                        ptl_tiles[qc] = t
                    else:
                        ptp_tiles[qc] = t
                ps = ps_st.tile([P, 2, NQ], f32, tag="st", name="st_ps")
                for kh in range(2):
                    kk = 2 * j + kh
                    for e in (0, 2):
                        if half == 0:
                            lhsT = kt_loc[:, e : e + 2, kk * P : (kk + 1) * P]
                        else:
                            lhsT = kt_rem[:, e : e + 2, kk * P : (kk + 1) * P]
                        nc.tensor.matmul(
                            ps[:, kh, :],
                            lhsT=lhsT,
                            rhs=qt_sb[:, e : e + 2, qsl],
                            start=(e == 0),
                            stop=(e == 2),
                            perf_mode=DR,
                        )
                pt_pair = pt_half(qc, half)[:, 2 * j : 2 * j + 2, :]
                nc.scalar.activation(out=pt_pair, in_=ps, func=AF.Exp, scale=SCALE)
                # Interleaved work (AV or Q-projection matmuls) is emitted
                # after the exp so the ACT queue stays a pure exp stream.
                if av is not None:
                    av(j)

            def rowsum(qc):
                # Rowsums on the PE: 16 all-ones fp8 DoubleRow matmuls over
                # the chunk's PT pairs, accumulated in one PSUM bank (the
                # result is partition-replicated), then the reciprocal.
                # Emitted right after P(qc): the last pair's exp completes
                # ~2us into the 3.5us matmul burst, so the PE barely waits.
                rs_ps = ps_rs.tile([P, NQ], f32, tag="rs", name=f"rs{qc}")
                for p in range(2 * NPAIR):
                    if p < NPAIR:
                        prhs = ptl_tiles[qc][:, 2 * p : 2 * p + 2, :]
                    else:
                        prhs = ptp_tiles[qc][:, 2 * (p - NPAIR) : 2 * (p - NPAIR) + 2, :]
                    nc.tensor.matmul(
                        rs_ps,
                        lhsT=ones_f8w,
                        rhs=prhs,
                        start=(p == 0),
                        stop=(p == 2 * NPAIR - 1),
                        perf_mode=DR,
                    )
                recips[qc] = outp.tile(
                    [P, NQ], f32, tag="recip", bufs=2, name=f"recip{qc}"
                )
                nc.vector.reciprocal(recips[qc], rs_ps)

            def bias2_compute():
                # bias2 = bv + (s_local + s_partner) / S on the (idle)
                # GpSimd engine: these [128, 4] ops are tiny even at Pool
                # throughput, and their wait on the s AllGather (~110us)
                # cannot back up the DVE mult/reciprocal stream.
                nc.gpsimd.tensor_add(s_a, s_a, s_b)
                for et in range(ET):
                    nc.gpsimd.tensor_scalar(
                        out=bias2[:, et : et + 1],
                        in0=s_a[:, et : et + 1],
                        scalar1=1.0 / S,
                        scalar2=bv_sb[:, et : et + 1],
                        op0=ALU.mult,
                        op1=ALU.add,
                    )

            av_state = {}

            def av_evac(qc, e, ops, halves=1):
                # Normalize on the DVE, bias-add on ACT (emitted after the
                # slot's exp, so it never sits in front of pending exps; the
                # ACT stream has ~10us of slack per AV-carrying block).
                # `halves=2` pipelines the mult/bias/DMA chain in 256-col
                # pieces — used on the final chunk to shorten the tail.
                tmp = outp.tile([P, NQ], f32, tag="tmp", bufs=3)
                hw = NQ // halves
                for hh in range(halves):
                    hs = slice(hh * hw, (hh + 1) * hw)
                    qsl = slice(qc * NQ + hh * hw, qc * NQ + (hh + 1) * hw)
                    nc.vector.tensor_mul(tmp[:, hs], ops[:, hs], recips[qc][:, hs])
                    nc.scalar.activation(
                        out=tmp[:, hs],
                        in_=tmp[:, hs],
                        func=AF.Identity,
                        bias=bias2[:, e : e + 1],
                        scale=1.0,
                    )
                    (nc.sync if (e + hh) % 2 == 0 else nc.gpsimd).dma_start(
                        out=ot[e * P : (e + 1) * P, qsl], in_=tmp[:, hs]
                    )

            def av_mms(qc, e, ops, p0, p1):
                for p in range(p0, p1):
                    if p < NPAIR:
                        vlhsT = v_loc[:, 2 * p : 2 * p + 2, e * P : (e + 1) * P]
                        prhs = ptl_tiles[qc][:, 2 * p : 2 * p + 2, :]
                    else:
                        pp = p - NPAIR
                        vlhsT = v_rem[:, 2 * pp : 2 * pp + 2, e * P : (e + 1) * P]
                        prhs = ptp_tiles[qc][:, 2 * pp : 2 * pp + 2, :]
                    nc.tensor.matmul(
                        ops,
                        lhsT=vlhsT,
                        rhs=prhs,
                        start=(p == 0),
                        stop=(p == 2 * NPAIR - 1),
                        perf_mode=DR,
                    )

            def av_slot(qc):
                # 8 AV matmuls per score slot, e-major: e-group e occupies
                # slots 2e (pairs 0-7) and 2e+1 (pairs 8-15 + evacuation).
                def fn(j):
                    e = j // 2
                    if j % 2 == 0:
                        av_state[qc] = ps_ot.tile(
                            [P, NQ], f32, tag="ot", name=f"av{qc}e{e}"
                        )
                        av_mms(qc, e, av_state[qc], 0, NPAIR)
                    else:
                        av_mms(qc, e, av_state[qc], NPAIR, 2 * NPAIR)
                        av_evac(qc, e, av_state[qc], halves=(2 if qc == QC - 1 else 1))
                return fn

            def q_slot(qc):
                # Q-projection for chunk qc rides in the previous local
                # score block, 2 matmuls per slot (e = j//2), with DVE
                # evacuation so the ACT exp stream is untouched.
                def fn(j):
                    e = j // 2
                    if j % 2 == 0:
                        q_ps[qc] = ps_ot.tile([P, NQ], f32, tag="ot", name=f"q{qc}e{e}")
                        q_mms(qc, e, 0, 2, q_ps[qc])
                    else:
                        q_mms(qc, e, 2, DT, q_ps[qc])
                        nc.vector.tensor_scalar(
                            out=qt_sb[:, e, qc * NQ : (qc + 1) * NQ],
                            in0=q_ps[qc],
                            scalar1=bq_sb[:, e : e + 1],
                            scalar2=None,
                            op0=ALU.add,
                        )
                return fn

            # Local-half score blocks, with Q chunks 1-3 interleaved.
            for qc in range(QC):
                av = q_slot(qc + 1) if qc < QC - 1 else None
                for j in range(NPAIR):
                    slot(qc, 0, j, av=av)
            # Partner-half blocks: P0 bare; av(qc) rides inside P(qc+1);
            # each chunk's rowsum matmul burst follows its last score block.
            for j in range(NPAIR):
                slot(0, 1, j)
            rowsum(0)
            bias2_compute()
            for qc in range(1, QC):
                avfn = av_slot(qc - 1)
                for j in range(NPAIR):
                    slot(qc, 1, j, av=avfn)
                rowsum(qc)
            # Chunk 3's AV tail runs after the last score block.
            avfn = av_slot(QC - 1)
            for j in range(2 * ET):
                avfn(j)

    _split_excess_waits(nc, mybir)
    return nc


def _get_nc():
    if "nc" not in _CACHE:
        _CACHE["nc"] = _build_nc()
    return _CACHE["nc"]


def _make_in_maps(x, Wq, bq, Wk, bk, Wv, bv):
    bf16 = ml_dtypes.bfloat16
    def _retile(W):
        # [P, DT*D] with w_re[p, d*D+j] = W.T[d*P+p, j]: per-partition
        # contiguous 4KB runs so the whole matrix loads as one DMA.
        return np.ascontiguousarray(
            W.T.reshape(DT, P, D).transpose(1, 0, 2).reshape(P, DT * D)
        ).astype(bf16)

    wqT = _retile(Wq)
    wkT = _retile(Wk)
    wvT = _retile(Wv)
    bqp = np.ascontiguousarray(bq.reshape(ET, P).T).astype(np.float32)
    bkp = np.ascontiguousarray(bk.reshape(ET, P).T).astype(np.float32)
    bvp = np.ascontiguousarray(bv.reshape(ET, P).T).astype(np.float32)
    in_maps = []
    for c in range(NCORES):
        b, h = divmod(c, 2)
        # Local half of x[b].T: both this core's query columns and its K/V
        # half (they are the same row range by construction).
        xTl = np.ascontiguousarray(x[b, h * SQ : (h + 1) * SQ, :].T).astype(bf16)
        # Partner-half row bases into the rank-ordered AllGather outputs.
        poff = np.array(
            [[(1 - h) * ET * P, (1 - h) * HKT * P]], dtype=np.uint32
        )
        in_maps.append(
            {
                "xT": xTl,
                "poff": poff,
                "wqT": wqT,
                "wkT": wkT,
                "wvT": wvT,
                "bq": bqp,
                "bk": bkp,
                "bv": bvp,
            }
        )
    return in_maps


def _run(in_maps, **kwargs):
    from concourse.bass_utils import run_bass_kernel_spmd

    nc = _get_nc()
    return run_bass_kernel_spmd(nc, in_maps, core_ids=list(range(NCORES)), **kwargs)


def kernel(x, Wq, bq, Wk, bk, Wv, bv):
    x = np.asarray(x, dtype=np.float32)
    Wq = np.asarray(Wq, dtype=np.float32)
    Wk = np.asarray(Wk, dtype=np.float32)
    Wv = np.asarray(Wv, dtype=np.float32)
    bq = np.asarray(bq, dtype=np.float32)
    bk = np.asarray(bk, dtype=np.float32)
    bv = np.asarray(bv, dtype=np.float32)

    res = _run(_make_in_maps(x, Wq, bq, Wk, bk, Wv, bv))
    out = np.empty((B, S, D), dtype=np.float32)
    for c in range(NCORES):
        b, h = divmod(c, 2)
        out[b, h * SQ : (h + 1) * SQ, :] = np.asarray(res.results[c]["ot"]).T
    return out


# revision 33
# speedup vs baseline: 1.0522x; 1.0129x over previous
"""BertSelfAttention (B=4, S=4096, D=512) on 8 TRN2 NeuronCores.

Sharding: core c handles batch b = c//2 and query-row half h = c%2
(2048 q rows). K/V are computed on-core for the full 4096 keys of that
batch (halves exchanged within each core pair), avoiding big collectives.

Layout trick: everything is computed transposed so no on-device
transposes are needed:
  QT[e, q] = Wq @ x.T          (lhsT = WqT chunks, rhs = xT chunks)
  KT[e, k] = Wk @ x.T
  V [k, e] = x @ Wv.T          (lhsT = xT chunks,  rhs = WvT)
  ST[k, q] = K Q.T             (lhsT = KT chunks,  rhs = QT)   -> exp -> PT
  OT[e, q] = V.T P.T           (lhsT = V chunks,   rhs = PT)
Softmax runs without max-subtraction (scores are ~N(0, 0.3^2), so exp
cannot overflow and the result is mathematically identical).

Precision: projections run in bf16 (fp32 PSUM); Q/K/V/P are quantized
to fp8e4 (e4m3) and the two big matmuls run as fp8 DoubleRow matmuls
(256-deep contraction per pass = 2x bf16 MAC rate on HW). The rank-1
component of the V-quantization error (softmax rows sum to one) is
corrected by adding (colsum(V_f32) - colsum(V_fp8))/S to the output
bias; colsum(V_f32) accumulates on the DVE during the V projection and
colsum(V_fp8) is an all-ones fp8 matmul, so the correction costs ~2us.

Schedule (trace-driven rewrite of the 209.6us baseline; ~202.5us):
- v1 put the softmax row-sums on the DVE (2 adds/pair): the DVE fell
  ~9us behind the exp stream, and its rowsum -> cast -> ones-matmul ->
  3.4us-reciprocal chain stalled the PE twice (8.9us + 4.3us) with
  clock-droop penalties on each restart. Here the row-sums are 16
  all-ones fp8 DoubleRow matmuls per chunk (+216ns/pair on the PE,
  partition-replicated result in one PSUM bank), emitted right after
  the chunk's partner score block; the reciprocal follows immediately
  and the DVE stays ~60% idle. (GpSimd tensor ops measured 3-15x
  slower than the cost model — eff ~0.05 for tensor_scalar — so
  offloading element work to Pool is a dead end on this HW.)
- AV matmuls for q-chunk qc are interleaved per-pair into the NEXT
  chunk's partner-half score block (8 av matmuls per score slot,
  e-major), so the PE runs back-to-back instead of alternating
  exp-paced score stretches with pure-AV stretches. Evacuation:
  DVE normalize-multiply, ACT bias-add (emitted after the slot's exp
  so the ACT queue stays exps-in-order), DMA out on alternating
  queues; the final chunk evacuates in 256-col halves to shorten the
  drain tail.
- K and V projections are fused per x column-chunk (4 KT e-tiles + 4 V
  k-tiles per chunk, ~7us of PE work per 512KB of arriving x), so the
  PE rides just behind the input DMA stream instead of stalling on it.
  Q chunks 1-3 ride inside the local score blocks (2 matmuls/slot,
  DVE evacuation); Q chunk 0 projects before the colsum matmuls so its
  ACT evacuations overlap them instead of gating the first score block.
- Inputs load via the 3 DMA-capable queues (SP/GpSimd/ACT) as whole-
  tensor transfers with 4KB-contiguous runs per partition (weights are
  host-retiled to [128, 4x512] blocks), sustaining ~330GB/s vs ~250
  for 1KB column-sliced loads; wk+wv first, then the x d-chunks; ~144
  throwaway matmuls warm the PE clock gate during the DMA wait.
  bias2 computes on GpSimd so its wait on the late s AllGather cannot
  back up the DVE mult/reciprocal stream.
- Collective order matters: KT, then V, then the tiny s exchange —
  inserting s before V costs a full collective-startup latency on the
  1MB V gather and starves the AV phase (measured +37us).
- PSUM: 2x2-bank score tiles, a 3-buffer [128,512] pool shared by AV
  accumulators and Q-projection tiles, and 1 rowsum bank (8/8 banks).
"""

import sys

for _p in ("/opt/trn_rl_repo", "/root/.axon_site/_ro/trn_rl_repo"):
    if _p not in sys.path:
        sys.path.append(_p)

import numpy as np
import ml_dtypes

B, S, D = 4, 4096, 512
NCORES = 8
SQ = S // 2  # query rows per core
P = 128
NQ = 512  # q-chunk width (moving free dim)
DT = D // P  # 4 contraction chunks for d
ET = D // P  # 4 e tiles
KTI = S // P  # 32 k tiles
QC = SQ // NQ  # 4 q chunks per core
HKT = KTI // 2  # 16 local k-tiles per core
HS = S // 2  # 2048 local keys per core
NPAIR = HKT // 2  # 8 k-tile pairs per half-block
SCALE = 1.0 / float(np.sqrt(np.float32(D)))
NWARM = 144

_CACHE = {}


def _split_excess_waits(nc, mybir, max_waits=1):
    """This walrus build rejects instructions carrying more than a couple of
    sync waits. Cap every instruction at `max_waits`, spilling the rest onto
    same-engine InstNoOps inserted immediately before it (equivalent
    semantics: the engine's stream stalls at the nop instead)."""
    for f in nc.m.functions:
        for bb in f.blocks:
            old = list(bb.instructions)
            new = []
            for inst in old:
                si = inst.sync_info
                waits = list(si.on_wait) if si is not None and si.on_wait else []
                if len(waits) > max_waits:
                    keep = waits[-max_waits:]
                    excess = waits[:-max_waits]
                    for i in range(0, len(excess), max_waits):
                        nop = mybir.InstNoOp(
                            name=f"waitnop-{nc.next_id()}", ins=[], outs=[]
                        )
                        nop.engine = inst.engine
                        nop.sync_info = mybir.SyncInfo(
                            on_wait=excess[i : i + max_waits], on_update=[]
                        )
                        new.append(nop)
                    inst.sync_info = mybir.SyncInfo(
                        on_wait=keep,
                        on_update=list(si.on_update) if si.on_update else [],
                    )
                new.append(inst)
            if len(new) != len(old):
                bb.instructions[:] = new


def _build_nc():
    import concourse.bass as bass
    import concourse.mybir as mybir
    import concourse.tile as tile
    from contextlib import ExitStack

    bf = mybir.dt.bfloat16
    f32 = mybir.dt.float32
    f8 = mybir.dt.float8e4
    AF = mybir.ActivationFunctionType
    DR = mybir.MatmulPerfMode.DoubleRow
    ALU = mybir.AluOpType

    u32 = mybir.dt.uint32
    nc = bass.Bass()
    xT = nc.declare_dram_parameter("xT", [D, SQ], bf, isOutput=False)
    # Weights host-retiled to [P, DT*D]: w_re[p, d*D+j] = wT[d*P+p, j], so
    # each partition's data is one contiguous 4KB run and the whole matrix
    # loads as a single high-throughput DMA.
    wqT = nc.declare_dram_parameter("wqT", [P, DT * D], bf, isOutput=False)
    wkT = nc.declare_dram_parameter("wkT", [P, DT * D], bf, isOutput=False)
    wvT = nc.declare_dram_parameter("wvT", [P, DT * D], bf, isOutput=False)
    bqp = nc.declare_dram_parameter("bq", [P, ET], f32, isOutput=False)
    bkp = nc.declare_dram_parameter("bk", [P, ET], f32, isOutput=False)
    bvp = nc.declare_dram_parameter("bv", [P, ET], f32, isOutput=False)
    # Host-computed row bases into the AllGather outputs for the PARTNER
    # half (rank-dependent: (1-h)*512 + e*128 for KT, (1-h)*2048 + j*128
    # for V). Drives dynamic (register-offset) DMAs.
    poffp = nc.declare_dram_parameter("poff", [1, 2], u32, isOutput=False)
    ot = nc.declare_dram_parameter("ot", [D, SQ], f32, isOutput=True)

    with tile.TileContext(nc) as tc, ExitStack() as ctx:
        const_pool = ctx.enter_context(tc.tile_pool(name="const", bufs=1))
        persist = ctx.enter_context(tc.tile_pool(name="persist", bufs=1))
        outp = ctx.enter_context(tc.tile_pool(name="outp", bufs=2))
        xin_pool = ctx.enter_context(tc.tile_pool(name="xin", bufs=1))

        ones = const_pool.tile([P, P], bf, tag="ones")
        nc.vector.memset(ones, 1.0)
        ones_f8 = const_pool.tile([P, 1], f8, tag="ones8")
        nc.gpsimd.memset(ones_f8, 1.0)
        # Wide all-ones fp8 stationary for the DoubleRow rowsum matmuls
        # (partition-reduces a PT pair and replicates across partitions).
        ones_f8w = const_pool.tile([P, 2, P], f8, tag="ones8w")
        nc.gpsimd.memset(ones_f8w, 1.0)
        bq_sb = const_pool.tile([P, ET], f32, tag="bq")
        bk_sb = const_pool.tile([P, ET], f32, tag="bk")
        bv_sb = const_pool.tile([P, ET], f32, tag="bv")
        wq_sb = const_pool.tile([P, DT, D], bf, tag="wq", name="wq")
        wk_sb = const_pool.tile([P, DT, D], bf, tag="wk", name="wk")
        wv_sb = const_pool.tile([P, DT, D], bf, tag="wv", name="wv")
        # fp8 operand tiles for the DoubleRow matmuls; contraction-paired
        # chunks live in dim 1 so [:, e:e+2, cols] is a valid 3D AP.
        # One qt tile PER CHUNK: with a single shared tile, the interleaved
        # Q-chunk evacuations created false tile-level write-after-read
        # dependencies that serialized every score block behind the next
        # chunk's Q writes (~12us of hidden slot-cadence loss).
        qt_c = [
            persist.tile([P, ET, NQ], f8, tag=f"qt{qc}", name=f"qt{qc}")
            for qc in range(QC)
        ]
        # K/V k-order per core: [my half, partner half]. Separate tiles per
        # half so partner DMA-writes create no false deps on local reads.
        kt_loc = persist.tile([P, ET, HS], f8, tag="ktl", name="ktl")
        kt_rem = persist.tile([P, ET, HS], f8, tag="ktr", name="ktr")
        v_loc = persist.tile([P, HKT, D], f8, tag="vl", name="vl")
        v_rem = persist.tile([P, HKT, D], f8, tag="vr", name="vr")
        poff_sb = const_pool.tile([1, 2], u32, tag="poff")
        bias2 = const_pool.tile([P, ET], f32, tag="bias2")

        # ---- Phase 1+2: load inputs on 4 DMA queues, project K + local V,
        # AllGather the K/V halves within each core pair, project Q chunk 0.
        with (
            tc.tile_pool(name="psA", bufs=4, space="PSUM") as psA,
            tc.tile_pool(name="dram", bufs=1, space="DRAM") as dram,
        ):
            ktl_d = dram.tile([ET * P, HS], f8, tag="ktl_d")
            ktg_d = dram.tile([2 * ET * P, HS], f8, tag="ktg_d")
            vl_d = dram.tile([HKT * P, D], f8, tag="vl_d")
            vg_d = dram.tile([2 * HKT * P, D], f8, tag="vg_d")
            sl_d = dram.tile([1, D], f32, tag="sl_d")
            sg_d = dram.tile([2, D], f32, tag="sg_d")

            x_sb = [xin_pool.tile([P, HS], bf, tag=f"x{d}", name=f"x{d}") for d in range(DT)]
            # 3 parallel hardware DMA queues (SP, GpSimd, ACT — the only
            # DMA-capable engines); per queue: wk chunks first (the first
            # projection needs them), then x column-chunks in first-consumer
            # order, then the later-needed weights/biases.
            qeng = [nc.sync, nc.gpsimd, nc.scalar]
            # Whole-tensor transfers with 4KB-contiguous runs per partition:
            # wk + wv upfront (the fused K/V projection needs both), the
            # four x d-chunks (full 2048-col rows), then the late-needed wq.
            qeng[0].dma_start(
                out=wk_sb, in_=wkT[:, :].rearrange("p (d c) -> p d c", d=DT)
            )
            qeng[1].dma_start(
                out=wv_sb, in_=wvT[:, :].rearrange("p (d c) -> p d c", d=DT)
            )
            for d in range(DT):
                qeng[(2 + d) % 3].dma_start(
                    out=x_sb[d], in_=xT[d * P : (d + 1) * P, :]
                )
            qeng[1].dma_start(
                out=wq_sb, in_=wqT[:, :].rearrange("p (d c) -> p d c", d=DT)
            )
            qeng[1].dma_start(out=bk_sb, in_=bkp[:, :])
            qeng[2].dma_start(out=bq_sb, in_=bqp[:, :])
            qeng[2].dma_start(out=bv_sb, in_=bvp[:, :])
            qeng[0].dma_start(out=poff_sb, in_=poffp[:, :])

            # Warm the PE HAM clock gate (~3.4us of activity flips it from
            # 1.2 to 2.4 GHz) with throwaway matmuls while the first input
            # DMAs are still in flight.
            warm_ps = psA.tile([P, P], f32, tag="warm", name="warm_ps", bufs=1)
            for _ in range(NWARM):
                nc.tensor.matmul(warm_ps, lhsT=ones, rhs=ones, start=True, stop=True)

            # Fused K+V projection, one x column-chunk at a time: each
            # arriving 512-col x chunk unlocks ~7us of PE work (4 KT e-tiles
            # + 4 V k-tiles), so the PE never waits on the x DMA stream.
            # K bias fused on ACT evacuation; V copied to fp8 on ACT with
            # the DVE accumulating colsum(V_f32) for the rank-1 correction.
            vacc = outp.tile([P, D], f32, tag="vacc", bufs=1)
            for kc in range(QC):
                for e in range(ET):
                    ps = psA.tile([P, NQ], f32, tag="ps")
                    for d in range(DT):
                        nc.tensor.matmul(
                            ps,
                            lhsT=wk_sb[:, d, e * P : (e + 1) * P],
                            rhs=x_sb[d][:, kc * NQ : (kc + 1) * NQ],
                            start=(d == 0),
                            stop=(d == DT - 1),
                        )
                    nc.scalar.activation(
                        out=kt_loc[:, e, kc * NQ : (kc + 1) * NQ],
                        in_=ps,
                        func=AF.Identity,
                        bias=bk_sb[:, e : e + 1],
                        scale=1.0,
                    )
                for kk in range(4):
                    k = 4 * kc + kk
                    ps = psA.tile([P, D], f32, tag="ps")
                    for d in range(DT):
                        nc.tensor.matmul(
                            ps,
                            lhsT=x_sb[d][:, k * P : (k + 1) * P],
                            rhs=wv_sb[:, d, :],
                            start=(d == 0),
                            stop=(d == DT - 1),
                        )
                    nc.scalar.copy(out=v_loc[:, k, :], in_=ps)
                    if k == 0:
                        nc.vector.tensor_copy(out=vacc, in_=ps)
                    else:
                        nc.vector.tensor_add(vacc, vacc, ps)
                    qeng[k % 3].dma_start(out=vl_d[k * P : (k + 1) * P, :], in_=v_loc[:, k, :])
            for e in range(ET):
                qeng[2 - (e % 2)].dma_start(
                    out=ktl_d[e * P : (e + 1) * P, :], in_=kt_loc[:, e, :]
                )
            # Start the KT exchange immediately: the CC engine is idle and
            # the partner half gates the partner-score blocks.
            pairs = [[2 * i, 2 * i + 1] for i in range(NCORES // 2)]
            nc.gpsimd.collective_compute(
                "AllGather",
                mybir.AluOpType.bypass,
                replica_groups=pairs,
                ins=[ktl_d.opt()],
                outs=[ktg_d.opt()],
            )
            nc.gpsimd.collective_compute(
                "AllGather",
                mybir.AluOpType.bypass,
                replica_groups=pairs,
                ins=[vl_d.opt()],
                outs=[vg_d.opt()],
            )
            # Q chunk 0 projected here, before the colsum matmuls: its ACT
            # evacuations then overlap the colsum work instead of gating the
            # first score block. (Chunks 1-3 ride inside the score blocks.)
            for e in range(ET):
                ps = psA.tile([P, NQ], f32, tag="ps")
                for d in range(DT):
                    nc.tensor.matmul(
                        ps,
                        lhsT=wq_sb[:, d, e * P : (e + 1) * P],
                        rhs=x_sb[d][:, 0:NQ],
                        start=(d == 0),
                        stop=(d == DT - 1),
                    )
                nc.scalar.activation(
                    out=qt_c[0][:, e, :],
                    in_=ps,
                    func=AF.Identity,
                    bias=bq_sb[:, e : e + 1],
                    scale=1.0,
                )
            # s_local = colsum(V_f32) - colsum(V_fp8): partition-reduce vacc
            # with a ones-matmul; colsum the fp8 tiles with an all-ones fp8
            # DoubleRow matmul (exact f32 accumulation).
            vacc_bf = outp.tile([P, D], bf, tag="vacc_bf", bufs=1)
            nc.vector.tensor_copy(out=vacc_bf, in_=vacc)
            psc = psA.tile([1, D], f32, tag="c1", bufs=1)
            nc.tensor.matmul(psc, lhsT=ones[:, 0:1], rhs=vacc_bf, start=True, stop=True)
            # colsum(V_fp8) via the wide all-ones DoubleRow matmul: the
            # result is partition-replicated, row 0 is the exact f32 colsum.
            psv = psA.tile([P, D], f32, tag="c2", bufs=1)
            for k in range(0, HKT, 2):
                nc.tensor.matmul(
                    psv,
                    lhsT=ones_f8w,
                    rhs=v_loc[:, k : k + 2, :],
                    start=(k == 0),
                    stop=(k == HKT - 2),
                    perf_mode=DR,
                )
            sl_c = outp.tile([1, D], f32, tag="sl_c", bufs=1)
            nc.scalar.copy(out=sl_c, in_=psc)
            sl_sb = outp.tile([1, D], f32, tag="sl_sb", bufs=1)
            nc.vector.tensor_sub(sl_sb, sl_c, psv[0:1, :])
            nc.sync.dma_start(out=sl_d, in_=sl_sb)
            nc.gpsimd.collective_compute(
                "AllGather",
                mybir.AluOpType.bypass,
                replica_groups=pairs,
                ins=[sl_d.opt()],
                outs=[sg_d.opt()],
            )

            # Partner-half loads from the gather outputs, issued before the
            # Q projection so the transfers start the moment each gather
            # lands. The row base is rank-dependent, supplied by the host
            # via `poff` and applied as a dynamic (register) offset.
            SP = [mybir.EngineType.SP]
            kt_base = nc.values_load(
                poff_sb[0:1, 0:1], engines=SP,
                min_val=0, max_val=ET * P,
                skip_runtime_bounds_check=True,
            )
            nc.sync.dma_start(
                out=kt_rem,
                in_=ktg_d[bass.ds(kt_base, ET * P), :].rearrange(
                    "(e p) c -> p e c", p=P
                ),
            )
            v_base = nc.values_load(
                poff_sb[0:1, 1:2], engines=SP,
                min_val=0, max_val=HKT * P,
                skip_runtime_bounds_check=True,
            )
            nc.sync.dma_start(
                out=v_rem,
                in_=vg_d[bass.ds(v_base, HKT * P), :].rearrange(
                    "(j p) c -> p j c", p=P
                ),
            )
            s_a = outp.tile([P, ET], f32, tag="s_a", bufs=1)
            s_b = outp.tile([P, ET], f32, tag="s_b", bufs=1)
            nc.sync.dma_start(
                out=s_a, in_=sg_d[0:1, :].rearrange("r (et p) -> (r p) et", p=P)
            )
            nc.sync.dma_start(
                out=s_b, in_=sg_d[1:2, :].rearrange("r (et p) -> (r p) et", p=P)
            )

        # ---- Phase 3: attention ----
        with (
            tc.tile_pool(name="pt", bufs=1) as pt_pool,
            tc.tile_pool(name="ps_st", bufs=2, space="PSUM") as ps_st,
            tc.tile_pool(name="ps_ot", bufs=3, space="PSUM") as ps_ot,
            tc.tile_pool(name="ps_rs", bufs=1, space="PSUM") as ps_rs,
        ):
            ptl_tiles = {}
            ptp_tiles = {}
            recips = {}

            # Q projection: chunk 0 eagerly (ACT evacuation: the exp stream
            # hasn't started); chunks 1-3 interleaved into the local score
            # blocks with DVE evacuation.
            def q_mms(qc, e, d0, d1, ps):
                for d in range(d0, d1):
                    nc.tensor.matmul(
                        ps,
                        lhsT=wq_sb[:, d, e * P : (e + 1) * P],
                        rhs=x_sb[d][:, qc * NQ : (qc + 1) * NQ],
                        start=(d == 0),
                        stop=(d == DT - 1),
                    )

            def pt_half(qc, half):
                return ptl_tiles[qc] if half == 0 else ptp_tiles[qc]

            q_ps = {}

            def slot(qc, half, j, av=None):
                """One k-tile-pair score slot: 4 DR matmuls -> exp -> rowsum
                (alternating DVE/GpSimd). `av` optionally appends AV or
                Q-projection matmuls to the PE stream inside this slot."""
                if j == 0:
                    t = pt_pool.tile(
                        [P, HKT, NQ], f8,
                        tag=("ptl" if half == 0 else "ptp"),
                        name=f"pt{half}_{qc}",
                        bufs=(4 if half == 0 else 3),
                    )
                    if half == 0:
                        ptl_tiles[qc] = t
                    else:
                        ptp_tiles[qc] = t
                ps = ps_st.tile([P, 2, NQ], f32, tag="st", name="st_ps")
                for kh in range(2):
                    kk = 2 * j + kh
                    for e in (0, 2):
                        if half == 0:
                            lhsT = kt_loc[:, e : e + 2, kk * P : (kk + 1) * P]
                        else:
                            lhsT = kt_rem[:, e : e + 2, kk * P : (kk + 1) * P]
                        nc.tensor.matmul(
                            ps[:, kh, :],
                            lhsT=lhsT,
                            rhs=qt_c[qc][:, e : e + 2, :],
                            start=(e == 0),
                            stop=(e == 2),
                            perf_mode=DR,
                        )
                pt_pair = pt_half(qc, half)[:, 2 * j : 2 * j + 2, :]
                nc.scalar.activation(out=pt_pair, in_=ps, func=AF.Exp, scale=SCALE)
                # Interleaved work (AV or Q-projection matmuls) is emitted
                # after the exp so the ACT queue stays a pure exp stream.
                if av is not None:
                    av(j)

            def rowsum(qc):
                # Rowsums on the PE: 16 all-ones fp8 DoubleRow matmuls over
                # the chunk's PT pairs, accumulated in one PSUM bank (the
                # result is partition-replicated), then the reciprocal.
                # Emitted right after P(qc): the last pair's exp completes
                # ~2us into the 3.5us matmul burst, so the PE barely waits.
                rs_ps = ps_rs.tile([P, NQ], f32, tag="rs", name=f"rs{qc}")
                for p in range(2 * NPAIR):
                    if p < NPAIR:
                        prhs = ptl_tiles[qc][:, 2 * p : 2 * p + 2, :]
                    else:
                        prhs = ptp_tiles[qc][:, 2 * (p - NPAIR) : 2 * (p - NPAIR) + 2, :]
                    nc.tensor.matmul(
                        rs_ps,
                        lhsT=ones_f8w,
                        rhs=prhs,
                        start=(p == 0),
                        stop=(p == 2 * NPAIR - 1),
                        perf_mode=DR,
                    )
                recips[qc] = outp.tile(
                    [P, NQ], f32, tag="recip", bufs=2, name=f"recip{qc}"
                )
                nc.vector.reciprocal(recips[qc], rs_ps)

            def bias2_compute():
                # bias2 = bv + (s_local + s_partner) / S on the (idle)
                # GpSimd engine: these [128, 4] ops are tiny even at Pool
                # throughput, and their wait on the s AllGather (~110us)
                # cannot back up the DVE mult/reciprocal stream.
                nc.gpsimd.tensor_add(s_a, s_a, s_b)
                for et in range(ET):
                    nc.gpsimd.tensor_scalar(
                        out=bias2[:, et : et + 1],
                        in0=s_a[:, et : et + 1],
                        scalar1=1.0 / S,
                        scalar2=bv_sb[:, et : et + 1],
                        op0=ALU.mult,
                        op1=ALU.add,
                    )

            av_state = {}

            def av_evac(qc, e, ops, halves=1):
                # Normalize on the DVE, bias-add on ACT (emitted after the
                # slot's exp, so it never sits in front of pending exps; the
                # ACT stream has ~10us of slack per AV-carrying block).
                # `halves=2` pipelines the mult/bias/DMA chain in 256-col
                # pieces — used on the final chunk to shorten the tail.
                tmp = outp.tile([P, NQ], f32, tag="tmp", bufs=3)
                hw = NQ // halves
                for hh in range(halves):
                    hs = slice(hh * hw, (hh + 1) * hw)
                    qsl = slice(qc * NQ + hh * hw, qc * NQ + (hh + 1) * hw)
                    nc.vector.tensor_mul(tmp[:, hs], ops[:, hs], recips[qc][:, hs])
                    nc.scalar.activation(
                        out=tmp[:, hs],
                        in_=tmp[:, hs],
                        func=AF.Identity,
                        bias=bias2[:, e : e + 1],
                        scale=1.0,
                    )
                    (nc.sync if (e + hh) % 2 == 0 else nc.gpsimd).dma_start(
                        out=ot[e * P : (e + 1) * P, qsl], in_=tmp[:, hs]
                    )

            def av_mms(qc, e, ops, p0, p1):
                for p in range(p0, p1):
                    if p < NPAIR:
                        vlhsT = v_loc[:, 2 * p : 2 * p + 2, e * P : (e + 1) * P]
                        prhs = ptl_tiles[qc][:, 2 * p : 2 * p + 2, :]
                    else:
                        pp = p - NPAIR
                        vlhsT = v_rem[:, 2 * pp : 2 * pp + 2, e * P : (e + 1) * P]
                        prhs = ptp_tiles[qc][:, 2 * pp : 2 * pp + 2, :]
                    nc.tensor.matmul(
                        ops,
                        lhsT=vlhsT,
                        rhs=prhs,
                        start=(p == 0),
                        stop=(p == 2 * NPAIR - 1),
                        perf_mode=DR,
                    )

            def av_slot(qc):
                # 8 AV matmuls per score slot, e-major: e-group e occupies
                # slots 2e (pairs 0-7) and 2e+1 (pairs 8-15 + evacuation).
                def fn(j):
                    e = j // 2
                    if j % 2 == 0:
                        av_state[qc] = ps_ot.tile(
                            [P, NQ], f32, tag="ot", name=f"av{qc}e{e}"
                        )
                        av_mms(qc, e, av_state[qc], 0, NPAIR)
                    else:
                        av_mms(qc, e, av_state[qc], NPAIR, 2 * NPAIR)
                        av_evac(qc, e, av_state[qc], halves=(2 if qc == QC - 1 else 1))
                return fn

            def q_slot(qc):
                # Q-projection for chunk qc rides in the previous local
                # score block, 2 matmuls per slot (e = j//2), with DVE
                # evacuation so the ACT exp stream is untouched.
                def fn(j):
                    e = j // 2
                    if j % 2 == 0:
                        q_ps[qc] = ps_ot.tile([P, NQ], f32, tag="ot", name=f"q{qc}e{e}")
                        q_mms(qc, e, 0, 2, q_ps[qc])
                    else:
                        q_mms(qc, e, 2, DT, q_ps[qc])
                        nc.vector.tensor_scalar(
                            out=qt_c[qc][:, e, :],
                            in0=q_ps[qc],
                            scalar1=bq_sb[:, e : e + 1],
                            scalar2=None,
                            op0=ALU.add,
                        )
                return fn

            # Local-half score blocks, with Q chunks 1-3 interleaved.
            for qc in range(QC):
                av = q_slot(qc + 1) if qc < QC - 1 else None
                for j in range(NPAIR):
                    slot(qc, 0, j, av=av)
            # Partner-half blocks: P0 bare; av(qc) rides inside P(qc+1);
            # each chunk's rowsum matmul burst follows its last score block.
            for j in range(NPAIR):
                slot(0, 1, j)
            rowsum(0)
            bias2_compute()
            for qc in range(1, QC):
                avfn = av_slot(qc - 1)
                for j in range(NPAIR):
                    slot(qc, 1, j, av=avfn)
                rowsum(qc)
            # Chunk 3's AV tail runs after the last score block.
            avfn = av_slot(QC - 1)
            for j in range(2 * ET):
                avfn(j)

    _split_excess_waits(nc, mybir)
    return nc


def _get_nc():
    if "nc" not in _CACHE:
        _CACHE["nc"] = _build_nc()
    return _CACHE["nc"]


def _make_in_maps(x, Wq, bq, Wk, bk, Wv, bv):
    bf16 = ml_dtypes.bfloat16
    def _retile(W):
        # [P, DT*D] with w_re[p, d*D+j] = W.T[d*P+p, j]: per-partition
        # contiguous 4KB runs so the whole matrix loads as one DMA.
        return np.ascontiguousarray(
            W.T.reshape(DT, P, D).transpose(1, 0, 2).reshape(P, DT * D)
        ).astype(bf16)

    wqT = _retile(Wq)
    wkT = _retile(Wk)
    wvT = _retile(Wv)
    bqp = np.ascontiguousarray(bq.reshape(ET, P).T).astype(np.float32)
    bkp = np.ascontiguousarray(bk.reshape(ET, P).T).astype(np.float32)
    bvp = np.ascontiguousarray(bv.reshape(ET, P).T).astype(np.float32)
    in_maps = []
    for c in range(NCORES):
        b, h = divmod(c, 2)
        # Local half of x[b].T: both this core's query columns and its K/V
        # half (they are the same row range by construction).
        xTl = np.ascontiguousarray(x[b, h * SQ : (h + 1) * SQ, :].T).astype(bf16)
        # Partner-half row bases into the rank-ordered AllGather outputs.
        poff = np.array(
            [[(1 - h) * ET * P, (1 - h) * HKT * P]], dtype=np.uint32
        )
        in_maps.append(
            {
                "xT": xTl,
                "poff": poff,
                "wqT": wqT,
                "wkT": wkT,
                "wvT": wvT,
                "bq": bqp,
                "bk": bkp,
                "bv": bvp,
            }
        )
    return in_maps


def _run(in_maps, **kwargs):
    from concourse.bass_utils import run_bass_kernel_spmd

    nc = _get_nc()
    return run_bass_kernel_spmd(nc, in_maps, core_ids=list(range(NCORES)), **kwargs)


def kernel(x, Wq, bq, Wk, bk, Wv, bv):
    x = np.asarray(x, dtype=np.float32)
    Wq = np.asarray(Wq, dtype=np.float32)
    Wk = np.asarray(Wk, dtype=np.float32)
    Wv = np.asarray(Wv, dtype=np.float32)
    bq = np.asarray(bq, dtype=np.float32)
    bk = np.asarray(bk, dtype=np.float32)
    bv = np.asarray(bv, dtype=np.float32)

    res = _run(_make_in_maps(x, Wq, bq, Wk, bk, Wv, bv))
    out = np.empty((B, S, D), dtype=np.float32)
    for c in range(NCORES):
        b, h = divmod(c, 2)
        out[b, h * SQ : (h + 1) * SQ, :] = np.asarray(res.results[c]["ot"]).T
    return out
